# revision 2
# baseline (speedup 1.0000x reference)
"""GAT 2-layer GNN kernel for 8 Trainium2 NeuronCores — v2.

Structure (vs the v1 baseline):
  - Nodes partitioned into 8 shards of 6250; per-core node features +
    attention scalars packed into 256-byte table rows; AllGather replicates
    the table (column-sliced: only the bytes the edge phase reads travel,
    and the AllGather is issued in 4 block-range pieces overlapped with
    table production).
  - Edges in ELL layout keyed by dst: dst slot = SBUF partition, per-dst
    edge list split into a lo run (table row < 32768) and a hi run, laid
    out per-block-contiguously [b0.lo|b0.hi|b1.lo|b1.hi|...] so each
    block's chunks are one contiguous range.
  - h[src]/a_src[src] fetched per-edge with dma_gather. Descriptor
    generation (~7.8ns/idx on the Q7) is the kernel bottleneck, so each
    group's gather is split into ~16-chunk segments spread over all 4
    SWDGE queues; queues 1-3 generate asynchronously, queue 0 inline on
    the Pool engine, giving ~4x parallel descriptor generation.
  - a_dst[dst] is never gathered: it stays on-chip in the per-core table
    SBUF copy (tab1_sb/tab2_sb) written during production.
  - Vector work is batched group-wide ([128, gch, H] ops) instead of
    per-run; the segment softmax-aggregate is identity-lhsT PSUM matmuls
    per chunk as before.
"""

import os
import sys

sys.path.insert(0, "/opt/trn_rl_repo")

import numpy as np
import ml_dtypes

import concourse.bacc as bacc
import concourse.mybir as mybir
from concourse import tile
from concourse.bass_utils import run_bass_kernel_spmd
from concourse.masks import make_identity

bf16 = ml_dtypes.bfloat16

N_NODES = 50000
F_IN = 512
H1 = 8
HID = 8
D1 = H1 * HID  # 64
C2 = 40
N_CORES = 8
SHARD = N_NODES // N_CORES  # 6250
BLK = 128
NB = (SHARD + BLK - 1) // BLK  # 49
SPLIT = 32768
HIBASE = N_NODES - SPLIT  # 17232: hi-run base row; rows [HIBASE,32768) are flexible
SBG = 2  # blocks per super-group
SEG = 16  # max chunks per gather segment
NEG_SLOPE = 0.2
TROW = 128  # table row: 128 bf16 = 256 bytes
NPIECE = 4

f32 = mybir.dt.float32
bfl = mybir.dt.bfloat16
i16 = mybir.dt.int16

# table col layouts (bf16 col units)
T1_AS = (64, 80)   # a_src1: 8 x f32
T1_AD = (80, 96)   # a_dst1: 8 x f32 (local only, not gathered/AG'd)
T1_AGC = 80        # AllGather cols 0:80 (160B)
T2_AS = (40, 42)   # a_src2: 1 x f32
T2_AD = (42, 44)   # a_dst2: local only
T2_AGC = 42        # AllGather cols 0:42 (84B)

_CACHE = {}


def _install_ntff_hook():
    """Provide antenv.axon_hooks if the image lacks it (NTFF profiling)."""
    try:
        from antenv.axon_hooks import get_axon_ntff_profile_hook  # noqa: F401
        return
    except ImportError:
        pass
    import contextlib
    import ctypes
    import types

    so_path = "/opt/axon/libaxon_pjrt.so"
    try:
        lib = ctypes.CDLL(so_path)
    except OSError:
        return
    if not hasattr(lib, "axon_start_nrt_profile"):
        return
    lib.axon_start_nrt_profile.argtypes = [ctypes.POINTER(ctypes.c_int64),
                                           ctypes.c_size_t]
    lib.axon_start_nrt_profile.restype = ctypes.c_int64
    lib.axon_stop_nrt_profile.argtypes = [ctypes.c_char_p]
    lib.axon_stop_nrt_profile.restype = ctypes.c_int64

    @contextlib.contextmanager
    def _hook(output_dir, device_ids):
        import jax
        jax.devices()
        if device_ids:
            ids = (ctypes.c_int64 * len(device_ids))(*device_ids)
            rc = lib.axon_start_nrt_profile(ids, len(device_ids))
        else:
            rc = lib.axon_start_nrt_profile(None, 0)
        if rc != 0:
            raise RuntimeError(f"axon_start_nrt_profile rc={rc}")
        try:
            yield
        finally:
            n = lib.axon_stop_nrt_profile(str(output_dir).encode())
            print(f"ntff profile: {n} file(s) written to {output_dir}")

    import antenv
    mod = types.ModuleType("antenv.axon_hooks")
    mod.get_axon_ntff_profile_hook = lambda: _hook
    mod.set_axon_ntff_profile_hook = lambda h: None
    sys.modules["antenv.axon_hooks"] = mod
    antenv.axon_hooks = mod


def _ceil(a, b):
    return (a + b - 1) // b


def _running_count(k):
    """pos[i] = number of j<i with k[j]==k[i]; k is sorted."""
    n = len(k)
    if n == 0:
        return np.zeros(0, np.int64)
    starts = np.r_[0, np.flatnonzero(np.diff(k)) + 1]
    run_id = np.zeros(n, np.int64)
    run_id[starts[1:]] = 1
    run_id = np.cumsum(run_id)
    return np.arange(n) - starts[run_id]


QMAP = [int(c) for c in os.environ.get("K_QMAP", "1230")]


class LayerPlan:
    pass


def _plan_layer(src_row, dst_node, perm, slot_of):
    """ELL plan with flexible lo/hi split: rows < HIBASE must use the lo
    base (0), rows >= SPLIT must use the hi base (HIBASE), rows in
    [HIBASE, SPLIT) may use either; per-block quotas pack each block near
    its max-degree bound. perm/slot_of: the common slot permutation."""
    plan = LayerPlan()
    core = dst_node // SHARD
    local = dst_node - core * SHARD
    cls = np.ones(len(src_row), np.int64)          # 1 = flexible
    cls[src_row < HIBASE] = 0                      # must-lo
    cls[src_row >= SPLIT] = 2                      # must-hi

    klo_f = np.zeros((N_CORES, SHARD), np.int64)
    khi_f = np.zeros((N_CORES, SHARD), np.int64)
    deg = np.zeros((N_CORES, SHARD), np.int64)
    np.add.at(klo_f, (core, local), (cls == 0).astype(np.int64))
    np.add.at(khi_f, (core, local), (cls == 2).astype(np.int64))
    np.add.at(deg, (core, local), 1)

    order = perm[:, :SHARD]
    pad = np.zeros((N_CORES, NB * BLK), np.int64)
    def blockmax(x):
        p = pad.copy()
        p[:, :SHARD] = np.take_along_axis(x, order, 1)
        return p.reshape(N_CORES, NB, BLK).max(axis=(0, 2))
    A = blockmax(klo_f)
    B = blockmax(khi_f)
    D = blockmax(deg)
    tot = np.maximum(np.maximum(A, 1) + np.maximum(B, 1), D)
    nch_lo = np.clip((tot + 1) // 2, np.maximum(A, 1), tot - np.maximum(B, 1))
    nch_hi = tot - nch_lo

    # chunk layout + gather segments (same as before)
    lo_start = np.zeros(NB, np.int64)
    hi_start = np.zeros(NB, np.int64)
    groups = []
    goff = 0
    rrq = 0
    qload = {q: 0 for q in QMAP}
    for g in range(_ceil(NB, SBG)):
        blocks = list(range(g * SBG, min((g + 1) * SBG, NB)))
        ch = goff
        runs = []
        for b in blocks:
            lo_start[b] = ch
            ch += int(nch_lo[b])
            hi_start[b] = ch
            ch += int(nch_hi[b])
            runs.append((int(lo_start[b] - goff), int(nch_lo[b]),
                         int(hi_start[b] - goff), int(nch_hi[b])))
        gch = ch - goff
        raw = []
        for b, (l0, nl, h0, nh) in zip(blocks, runs):
            for r0, rn, ishi in ((l0, nl, 0), (h0, nh, 1)):
                c = r0
                while c < r0 + rn:
                    n = min(SEG, r0 + rn - c)
                    raw.append((c, n, ishi))
                    c += n
        # greedy least-loaded queue assignment (global load tracking)
        segs = []
        for (c, n, ishi) in sorted(raw, key=lambda x: -x[1]):
            q = min(QMAP, key=lambda qq: qload[qq])
            qload[q] += n
            segs.append((c, n, ishi, q))
        segs = [x for x in segs if x[3] != 0] + [x for x in segs if x[3] == 0]
        groups.append({"blocks": blocks, "goff": goff, "gch": gch,
                       "runs": runs, "segs": segs})
        goff = ch
    total_ch = goff

    Klo_of_blk = nch_lo  # per-block lo quota
    idx_streams, mask_streams = [], []
    for c in range(N_CORES):
        sel = core == c
        e_row = src_row[sel]
        e_loc = local[sel]
        e_cls = cls[sel]
        e_slot = slot_of[c, e_loc]
        e_blk = e_slot // BLK

        o = np.lexsort((e_cls, e_slot))
        r = _running_count(e_slot[o])
        blk_o = e_blk[o]
        is_lo_o = (e_cls[o] == 0) | ((e_cls[o] == 1) & (r < Klo_of_blk[blk_o]))
        # lo edges come first within a slot, so pos_lo = r; pos_hi = r - klo_c
        klo_c = klo_f[c] + np.clip(
            Klo_of_blk[(slot_of[c] // BLK)] - klo_f[c], 0,
            deg[c] - klo_f[c] - khi_f[c])
        pos = np.where(is_lo_o, r, r - klo_c[e_loc[o]])
        chunk = np.where(is_lo_o, lo_start[blk_o], hi_start[blk_o]) + pos
        slots = chunk * BLK + (e_slot[o] % BLK)

        idx = np.zeros(total_ch * BLK, np.int16)
        mask = np.zeros(total_ch * BLK, np.float32)
        idx[slots] = (e_row[o] - np.where(is_lo_o, 0, HIBASE)).astype(np.int16)
        mask[slots] = 1.0

        idx_w = np.tile(idx.reshape(total_ch * 8, 16).T, (8, 1)).copy()
        mask_w = mask.reshape(total_ch, BLK).T.astype(bf16).copy()
        idx_streams.append(idx_w)
        mask_streams.append(mask_w)

    plan.nch_lo = nch_lo
    plan.nch_hi = nch_hi
    plan.groups = groups
    plan.total_ch = total_ch
    plan.idx_streams = idx_streams
    plan.mask_streams = mask_streams
    plan.perm = perm
    return plan


def _prep(edge_index):
    """One common slot permutation; x is host-permuted into slot order, so
    both layers share one table row space and ONE plan (identical streams)."""
    src = np.asarray(edge_index[0], dtype=np.int64)
    dst = np.asarray(edge_index[1], dtype=np.int64)
    # self-loops are handled analytically on-chip, not in the edge streams

    row_pm = _row_of_piece_major()
    s_core = src // SHARD
    s_local = src - s_core * SHARD
    core = dst // SHARD
    local = dst - core * SHARD
    deg = np.zeros((N_CORES, SHARD), np.int64)
    np.add.at(deg, (core, local), 1)

    order = np.argsort(-deg, axis=1, kind="stable")
    for _ in range(2):
        slot_of = np.zeros((N_CORES, SHARD), np.int64)
        for c in range(N_CORES):
            slot_of[c, order[c]] = np.arange(SHARD)
        srow = row_pm[s_core, slot_of[s_core, s_local]]
        klo_f = np.zeros((N_CORES, SHARD), np.int64)
        np.add.at(klo_f, (core, local), (srow < HIBASE).astype(np.int64))
        order = np.lexsort((-klo_f, -deg), axis=-1)

    perm = np.full((N_CORES, NB * BLK), -1, np.int64)
    slot_of = np.zeros((N_CORES, SHARD), np.int64)
    for c in range(N_CORES):
        perm[c, :SHARD] = order[c]
        slot_of[c, order[c]] = np.arange(SHARD)
    src_row = row_pm[s_core, slot_of[s_core, s_local]]
    plan = _plan_layer(src_row, dst, perm, slot_of)
    return plan, plan


def _piece_bounds():
    """NPIECE block ranges aligned to group (SBG) boundaries."""
    ngroups = _ceil(NB, SBG)
    gb = [round(i * ngroups / NPIECE) for i in range(NPIECE + 1)]
    return [(gb[i], gb[i + 1], min(gb[i] * SBG, NB), min(gb[i + 1] * SBG, NB))
            for i in range(NPIECE)]


def _piece_rows():
    """Local-row ranges per piece and piece-major t_full bases (in rows)."""
    pr = [min(be * BLK, SHARD) for (_, _, _, be) in _piece_bounds()]
    pr = [0] + pr
    base = [N_CORES * r for r in pr]
    return pr, base


def _row_of_piece_major():
    """row_of[c, local] = piece-major t_full row of core c's local row."""
    pr, base = _piece_rows()
    row_of = np.zeros((N_CORES, SHARD), np.int64)
    for p in range(NPIECE):
        r0, r1 = pr[p], pr[p + 1]
        ln = r1 - r0
        for c in range(N_CORES):
            row_of[c, r0:r1] = base[p] + c * ln + np.arange(ln)
    return row_of


def _dma_blocks_out(nc, shard_dram, tab_sb, bs, be):
    """DMA tab_sb[:, bs:be, :] to shard_dram rows [bs*BLK, min(be*BLK, SHARD))."""
    r0 = bs * BLK
    r1 = min(be * BLK, SHARD)
    full = (r1 - r0) // BLK  # full blocks
    if full > 0:
        nc.scalar.dma_start(
            out=shard_dram[r0:r0 + full * BLK, :].rearrange(
                "(b p) c -> p b c", p=BLK, b=full),
            in_=tab_sb[:, bs:bs + full, :])
    rem = (r1 - r0) - full * BLK
    if rem > 0:
        nc.scalar.dma_start(out=shard_dram[r0 + full * BLK:r1, :],
                          in_=tab_sb[0:rem, bs + full, :])


AG_MODE = int(os.environ.get("K_AG_MODE", "3"))
GQMAP = [int(c) for c in os.environ.get("K_QMAP", "1230")]


def _ag_piece(nc, shard_dram, full_dram, bs, be, ncols, rg):
    """AllGather rows [bs*BLK, min(be*BLK, SHARD)) x cols [0:ncols)."""
    if AG_MODE == 0:
        return
    r0 = bs * BLK
    r1 = min(be * BLK, SHARD)
    if AG_MODE == 3:
        pr, base = _piece_rows()
        p = pr.index(r0)
        assert pr[p + 1] == r1
        nc.gpsimd.collective_compute(
            "AllGather", mybir.AluOpType.bypass, replica_groups=rg,
            ins=[shard_dram[r0:r1, :].opt()],
            outs=[full_dram[base[p]:base[p + 1], :].opt()])
        return
    full_v = full_dram.ap().rearrange("(r n) c -> r n c", r=N_CORES)
    if AG_MODE == 2:
        ncols = TROW
    nc.gpsimd.collective_compute(
        "AllGather", mybir.AluOpType.bypass, replica_groups=rg,
        ins=[shard_dram[r0:r1, 0:ncols]],
        outs=[full_v[:, r0:r1, 0:ncols]])


def _ag_whole(nc, shard_dram, full_dram, rg):
    if AG_MODE != 0:
        return
    nc.gpsimd.collective_compute(
        "AllGather", mybir.AluOpType.bypass, replica_groups=rg,
        ins=[shard_dram.ap().opt()], outs=[full_dram.ap().opt()])


def _build(plan1, plan2):
    nc = bacc.Bacc("TRN2", target_bir_lowering=False, debug=False,
                   num_devices=N_CORES, num_swdge_queues=4)

    NPADROWS = NB * BLK  # 6272
    xT_ext = nc.declare_dram_parameter("xT", [F_IN, NPADROWS], bfl, isOutput=False)
    w1_ext = nc.declare_dram_parameter("w1r", [128, 4 * D1], bfl, isOutput=False)
    w2_ext = nc.declare_dram_parameter("w2", [D1, C2], bfl, isOutput=False)
    a1s_ext = nc.declare_dram_parameter("a1srep", [128, D1], f32, isOutput=False)
    a1d_ext = nc.declare_dram_parameter("a1drep", [128, D1], f32, isOutput=False)
    a2s_ext = nc.declare_dram_parameter("a2srep", [128, C2], f32, isOutput=False)
    a2d_ext = nc.declare_dram_parameter("a2drep", [128, C2], f32, isOutput=False)
    b1_ext = nc.declare_dram_parameter("b1rep", [128, D1], f32, isOutput=False)
    b2_ext = nc.declare_dram_parameter("b2rep", [128, C2], f32, isOutput=False)
    idx1_ext = nc.declare_dram_parameter("idx1", [128, plan1.total_ch * 8], i16,
                                         isOutput=False)
    msk1_ext = nc.declare_dram_parameter("msk1", [128, plan1.total_ch], bfl,
                                         isOutput=False)
    out_ext = nc.declare_dram_parameter("out", [NB * BLK, C2], f32, isOutput=True)

    t1_shard = nc.dram_tensor("t1_shard", [SHARD, TROW], bfl)
    t1_full = nc.dram_tensor("t1_full", [N_NODES, TROW], bfl, addr_space="Shared")
    t2_shard = nc.dram_tensor("t2_shard", [SHARD, TROW], bfl)
    t2_full = nc.dram_tensor("t2_full", [N_NODES, TROW], bfl, addr_space="Shared")

    rg = [list(range(N_CORES))]
    pieces = _piece_bounds()

    with tile.TileContext(nc) as tc:
        with tc.tile_pool(name="const", bufs=1) as cpool:
            ident = cpool.tile([128, 128], bfl)
            make_identity(nc, ident[:, :])
            a1s_t = cpool.tile([128, D1], f32)
            nc.sync.dma_start(out=a1s_t[:, :], in_=a1s_ext[:, :])
            a1d_t = cpool.tile([128, D1], f32)
            nc.sync.dma_start(out=a1d_t[:, :], in_=a1d_ext[:, :])
            a2s_t = cpool.tile([128, C2], f32)
            nc.sync.dma_start(out=a2s_t[:, :], in_=a2s_ext[:, :])
            a2d_t = cpool.tile([128, C2], f32)
            nc.sync.dma_start(out=a2d_t[:, :], in_=a2d_ext[:, :])
            b1_t = cpool.tile([128, D1], f32)
            nc.sync.dma_start(out=b1_t[:, :], in_=b1_ext[:, :])
            b2_t = cpool.tile([128, C2], f32)
            nc.sync.dma_start(out=b2_t[:, :], in_=b2_ext[:, :])
            w2_t = cpool.tile([D1, C2], bfl)
            nc.sync.dma_start(out=w2_t[:, :], in_=w2_ext[:, :])
            tab1_sb = cpool.tile([128, NB, TROW], bfl)
            tab2_sb = cpool.tile([128, NB, TROW], bfl)
            nc.vector.memset(tab1_sb[:, :, :], 0.0)
            nc.vector.memset(tab2_sb[:, :, :], 0.0)
            h1f = cpool.tile([128, NB, D1], f32)
            h2f = cpool.tile([128, NB, C2], f32)

            # ---- Phase A: h1 = x @ W1 + attention scalars, piecewise AG1
            with tc.tile_pool(name="phA", bufs=2) as apool, \
                 tc.tile_pool(name="phA_ps", bufs=4, space="PSUM") as apsum:
                w1_t = apool.tile([128, 4, D1], bfl, tag="w1")
                nc.sync.dma_start(out=w1_t[:, :, :], in_=w1_ext[:, :])
                xk = []
                for k in range(4):
                    xt = apool.tile([128, NPADROWS], bfl, tag=f"xk{k}")
                    nc.sync.dma_start(out=xt[:, :],
                                      in_=xT_ext[k * 128:(k + 1) * 128, :])
                    xk.append(xt)
                for (g0, g1, bs, be) in pieces:
                    for b in range(bs, be):
                        hps = apsum.tile([128, D1], f32, tag="hps")
                        for k in range(4):
                            nc.tensor.matmul(
                                hps[:, :],
                                lhsT=xk[k][:, b * BLK:(b + 1) * BLK],
                                rhs=w1_t[:, k, :], start=(k == 0), stop=(k == 3))
                        nc.scalar.activation(
                            out=tab1_sb[:, b, 0:D1], in_=hps[:, :],
                            func=mybir.ActivationFunctionType.Copy)
                        nc.scalar.activation(
                            out=h1f[:, b, :], in_=hps[:, :],
                            func=mybir.ActivationFunctionType.Copy)
                    nb = be - bs
                    for a_t, sl in ((a1s_t, T1_AS), (a1d_t, T1_AD)):
                        tmp = apool.tile([128, nb, D1], f32, tag="atmp")
                        nc.vector.tensor_tensor(
                            out=tmp[:, :, :], in0=h1f[:, bs:be, :],
                            in1=a_t[:, None, :].to_broadcast([128, nb, D1]),
                            op=mybir.AluOpType.mult)
                        nc.vector.tensor_reduce(
                            out=tab1_sb[:, bs:be, sl[0]:sl[1]].bitcast(f32),
                            in_=tmp[:, :, :].rearrange(
                                "p b (h c) -> p b h c", h=H1, c=HID),
                            axis=mybir.AxisListType.X, op=mybir.AluOpType.add)
                    _dma_blocks_out(nc, t1_shard, tab1_sb, bs, be)
                    _ag_piece(nc, t1_shard, t1_full, bs, be, T1_AGC, rg)

            _ag_whole(nc, t1_shard, t1_full, rg)

            _edge_phase(nc, tc, layer=1, table_full=t1_full,
                        idx_ext=idx1_ext, msk_ext=msk1_ext, ident=ident,
                        plan=plan1, pieces=pieces, bias=b1_t, w2_t=w2_t,
                        a2s_t=a2s_t, a2d_t=a2d_t, tab_sb=tab1_sb,
                        tab_out=tab2_sb, t_shard=t2_shard, t_full=t2_full,
                        rg=rg, out_ext=None, b2_t=None, h2f=h2f)

            _ag_whole(nc, t2_shard, t2_full, rg)

            _edge_phase(nc, tc, layer=2, table_full=t2_full,
                        idx_ext=idx1_ext, msk_ext=msk1_ext, ident=ident,
                        plan=plan2, pieces=pieces, bias=None, w2_t=None,
                        a2s_t=None, a2d_t=None, tab_sb=tab2_sb,
                        tab_out=None, t_shard=None, t_full=None,
                        rg=rg, out_ext=out_ext, b2_t=b2_t, h2f=None)

    nc.compile()
    return nc


def _edge_phase(nc, tc, layer, table_full, idx_ext, msk_ext, ident,
                plan, pieces, bias, w2_t, a2s_t, a2d_t, tab_sb,
                tab_out, t_shard, t_full, rg, out_ext, b2_t, h2f):
    if layer == 1:
        NH, CH, CC = H1, HID, D1
        asrc_sl, adst_sl = T1_AS, T1_AD
    else:
        NH, CH, CC = 1, C2, C2
        asrc_sl, adst_sl = T2_AS, T2_AD
    NCOL = CC + NH
    gmax = max(g["gch"] for g in plan.groups)

    with tc.tile_pool(name=f"self{layer}", bufs=1) as spool, \
         tc.tile_pool(name=f"e{layer}", bufs=2) as pool, \
         tc.tile_pool(name=f"e{layer}_ps", bufs=2, space="PSUM") as psum, \
         tc.tile_pool(name=f"e{layer}_ps2", bufs=2, space="PSUM") as psum2:
        # analytic self-loop term: w = exp(leakyrelu(a_src[d]+a_dst[d])),
        # num += w*h[d], den += w  (self-loops excluded from edge streams)
        es = spool.tile([128, NB, NH], f32)
        nc.vector.tensor_tensor(
            out=es[:, :, :],
            in0=tab_sb[:, :, asrc_sl[0]:asrc_sl[1]].bitcast(f32),
            in1=tab_sb[:, :, adst_sl[0]:adst_sl[1]].bitcast(f32),
            op=mybir.AluOpType.add)
        lrs = spool.tile([128, NB, NH], f32)
        nc.vector.scalar_tensor_tensor(
            out=lrs[:, :, :], in0=es[:, :, :], scalar=NEG_SLOPE,
            in1=es[:, :, :], op0=mybir.AluOpType.mult,
            op1=mybir.AluOpType.max)
        ws = spool.tile([128, NB, NH], f32)
        nc.scalar.activation(out=ws[:, :, :], in_=lrs[:, :, :],
                             func=mybir.ActivationFunctionType.Exp)
        sn = spool.tile([128, NB, CC], f32)
        nc.vector.tensor_tensor(
            out=sn[:, :, :].rearrange("p b (h c) -> p b h c", h=NH, c=CH),
            in0=tab_sb[:, :, 0:CC].rearrange("p b (h c) -> p b h c",
                                             h=NH, c=CH),
            in1=ws[:, :, :, None].to_broadcast([128, NB, NH, CH]),
            op=mybir.AluOpType.mult)
        piece_of_group = {}
        for pi, (g0, g1, bs, be) in enumerate(pieces):
            for g in range(g0, g1):
                piece_of_group[g] = pi if g == g1 - 1 else None

        for gi, grp in enumerate(plan.groups):
            goff, gch = grp["goff"], grp["gch"]
            idxg = pool.tile([128, gmax * 8], i16, tag="idxg")
            nc.sync.dma_start(out=idxg[:, 0:gch * 8],
                              in_=idx_ext[:, goff * 8:(goff + gch) * 8])
            mskg = pool.tile([128, gmax], bfl, tag="mskg")
            nc.sync.dma_start(out=mskg[:, 0:gch],
                              in_=msk_ext[:, goff:goff + gch])
            g_t = pool.tile([128, gmax, TROW], bfl, tag="gath")
            for (c0, n, ishi, q) in grp["segs"]:
                in_ap = (table_full[HIBASE:N_NODES, :] if ishi
                         else table_full[0:SPLIT, :])
                nc.gpsimd.dma_gather(
                    out_ap=g_t[:, c0:c0 + n, :], in_ap=in_ap,
                    idxs_ap=idxg[:, c0 * 8:(c0 + n) * 8],
                    num_idxs=n * BLK, num_idxs_reg=n * BLK,
                    elem_size=TROW, single_packet=False, queue_num=q)

            # group-wide attention pipeline
            e_t = pool.tile([128, gmax, NH], f32, tag="elog")
            for b, (l0, nl, h0, nh) in zip(grp["blocks"], grp["runs"]):
                rn = nl + nh
                nc.vector.tensor_tensor(
                    out=e_t[:, l0:l0 + rn, :],
                    in0=g_t[:, l0:l0 + rn,
                            asrc_sl[0]:asrc_sl[1]].bitcast(f32),
                    in1=tab_sb[:, b, adst_sl[0]:adst_sl[1]].bitcast(f32)
                        [:, None, :].to_broadcast([128, rn, NH]),
                    op=mybir.AluOpType.add)
            lr_t = pool.tile([128, gmax, NH], f32, tag="lrt")
            nc.vector.scalar_tensor_tensor(
                out=lr_t[:, 0:gch, :], in0=e_t[:, 0:gch, :], scalar=NEG_SLOPE,
                in1=e_t[:, 0:gch, :], op0=mybir.AluOpType.mult,
                op1=mybir.AluOpType.max)
            exf = pool.tile([128, gmax, NH], f32, tag="exf")
            nc.scalar.activation(out=exf[:, 0:gch, :], in_=lr_t[:, 0:gch, :],
                                 func=mybir.ActivationFunctionType.Exp)
            r_t = pool.tile([128, gmax, NCOL], bfl, tag="rmat")
            nc.vector.tensor_tensor(
                out=r_t[:, 0:gch, CC:NCOL], in0=exf[:, 0:gch, :],
                in1=mskg[:, 0:gch, None].to_broadcast([128, gch, NH]),
                op=mybir.AluOpType.mult)
            nc.vector.tensor_tensor(
                out=r_t[:, 0:gch, 0:CC].rearrange("p g (h c) -> p g h c",
                                                  h=NH, c=CH),
                in0=g_t[:, 0:gch, 0:CC].rearrange("p g (h c) -> p g h c",
                                                  h=NH, c=CH),
                in1=r_t[:, 0:gch, CC:NCOL, None].to_broadcast(
                    [128, gch, NH, CH]),
                op=mybir.AluOpType.mult)

            for b, (l0, nl, h0, nh) in zip(grp["blocks"], grp["runs"]):
                rn = nl + nh
                ps = psum.tile([128, NCOL], f32, tag="agg")
                for j in range(rn):
                    nc.tensor.matmul(ps[:, :], lhsT=ident[:, :],
                                     rhs=r_t[:, l0 + j, :], start=(j == 0),
                                     stop=(j == rn - 1))
                den = pool.tile([128, NH], f32, tag="den")
                nc.vector.scalar_tensor_tensor(
                    out=den[:, :], in0=ps[:, CC:NCOL], scalar=1e-16,
                    in1=ws[:, b, :], op0=mybir.AluOpType.add,
                    op1=mybir.AluOpType.add)
                recip = pool.tile([128, NH], f32, tag="recip")
                nc.vector.reciprocal(out=recip[:, :], in_=den[:, :])
                onum = pool.tile([128, CC], f32, tag="onum")
                nc.vector.tensor_tensor(out=onum[:, :], in0=ps[:, 0:CC],
                                        in1=sn[:, b, :],
                                        op=mybir.AluOpType.add)
                o_t = pool.tile([128, CC], f32, tag="outb")
                nc.vector.tensor_tensor(
                    out=o_t[:, :].rearrange("p (h c) -> p h c", h=NH, c=CH),
                    in0=onum[:, :].rearrange("p (h c) -> p h c", h=NH, c=CH),
                    in1=recip[:, :, None].to_broadcast([128, NH, CH]),
                    op=mybir.AluOpType.mult)

                if layer == 1:
                    obt = pool.tile([128, CC], f32, tag="outbt")
                    nc.vector.tensor_tensor(out=obt[:, :], in0=o_t[:, :],
                                            in1=bias[:, :],
                                            op=mybir.AluOpType.add)
                    ob = pool.tile([128, CC], bfl, tag="outbf")
                    nc.vector.tensor_scalar(out=ob[:, :], in0=obt[:, :],
                                            scalar1=0.0, scalar2=None,
                                            op0=mybir.AluOpType.max)
                    tps = psum2.tile([D1, 128], bfl, tag="tp")
                    nc.tensor.transpose(tps[:, :], ob[:, :], ident[:, :])
                    h1T = pool.tile([D1, 128], bfl, tag="h1T")
                    nc.vector.tensor_copy(out=h1T[:, :], in_=tps[:, :])
                    h2ps = psum2.tile([128, C2], f32, tag="h2")
                    nc.tensor.matmul(h2ps[:, :], lhsT=h1T[:, :], rhs=w2_t[:, :],
                                     start=True, stop=True)
                    nc.scalar.activation(out=tab_out[:, b, 0:C2],
                                         in_=h2ps[:, :],
                                         func=mybir.ActivationFunctionType.Copy)
                    nc.scalar.activation(out=h2f[:, b, :], in_=h2ps[:, :],
                                         func=mybir.ActivationFunctionType.Copy)
                else:
                    lg = pool.tile([128, C2], f32, tag="logits")
                    nc.vector.tensor_tensor(out=lg[:, :], in0=o_t[:, :],
                                            in1=b2_t[:, :],
                                            op=mybir.AluOpType.add)
                    negm = pool.tile([128, 1], f32, tag="negm")
                    nc.vector.tensor_reduce(out=negm[:, :], in_=lg[:, :],
                                            axis=mybir.AxisListType.X,
                                            op=mybir.AluOpType.max, negate=True)
                    ex = pool.tile([128, C2], f32, tag="sfex")
                    ssum = pool.tile([128, 1], f32, tag="ssum")
                    nc.scalar.activation(out=ex[:, :], in_=lg[:, :],
                                         func=mybir.ActivationFunctionType.Exp,
                                         bias=negm[:, :], accum_out=ssum[:, :])
                    lse = pool.tile([128, 1], f32, tag="lse")
                    nc.scalar.activation(out=lse[:, :], in_=ssum[:, :],
                                         func=mybir.ActivationFunctionType.Ln)
                    res = pool.tile([128, C2], f32, tag="res")
                    nc.vector.scalar_tensor_tensor(
                        out=res[:, :], in0=lg[:, :], scalar=negm[:, :],
                        in1=lse[:, :].to_broadcast([128, C2]),
                        op0=mybir.AluOpType.add, op1=mybir.AluOpType.subtract)
                    nc.scalar.dma_start(out=out_ext[b * BLK:(b + 1) * BLK, :],
                                      in_=res[:, :])

            if layer == 1:
                pi = piece_of_group.get(gi)
                if pi is not None:
                    g0, g1, bs, be = pieces[pi]
                    nb = be - bs
                    for a_t, sl in ((a2s_t, T2_AS), (a2d_t, T2_AD)):
                        tmp = pool.tile([128, nb, C2], f32, tag="a2tmp")
                        nc.vector.tensor_tensor(
                            out=tmp[:, :, :], in0=h2f[:, bs:be, :],
                            in1=a_t[:, None, :].to_broadcast([128, nb, C2]),
                            op=mybir.AluOpType.mult)
                        nc.vector.tensor_reduce(
                            out=tab_out[:, bs:be, sl[0]:sl[1]].bitcast(f32),
                            in_=tmp[:, :, :], axis=mybir.AxisListType.X,
                            op=mybir.AluOpType.add)
                    _dma_blocks_out(nc, t_shard, tab_out, bs, be)
                    _ag_piece(nc, t_shard, t_full, bs, be, T2_AGC, rg)


def _host_inputs(x, W1, att_src1, att_dst1, b1, W2, att_src2, att_dst2, b2,
                 plan1, plan2):
    NPADROWS = NB * BLK
    w1r = np.ascontiguousarray(
        np.asarray(W1, np.float32).reshape(4, 128, D1).transpose(1, 0, 2)
    ).reshape(128, 4 * D1).astype(bf16)
    rep = lambda v, n: np.tile(np.asarray(v, np.float32).reshape(1, n),
                               (128, 1)).astype(np.float32)
    x32 = np.asarray(x, np.float32)

    in_maps = []
    for c in range(N_CORES):
        xs = x32[c * SHARD:(c + 1) * SHARD][plan1.perm[c, :SHARD]]
        xT = np.zeros((F_IN, NPADROWS), bf16)
        xT[:, :SHARD] = xs.T.astype(bf16)
        in_maps.append({
            "xT": xT,
            "w1r": w1r,
            "w2": np.asarray(W2, np.float32).astype(bf16),
            "a1srep": rep(att_src1, D1),
            "a1drep": rep(att_dst1, D1),
            "a2srep": rep(att_src2, C2),
            "a2drep": rep(att_dst2, C2),
            "b1rep": rep(b1, D1),
            "b2rep": rep(b2, C2),
            "idx1": plan1.idx_streams[c],
            "msk1": plan1.mask_streams[c],
        })
    return in_maps


def kernel_run(inputs, trace=False):
    edge_index = inputs["edge_index"]
    plan1, plan2 = _prep(edge_index)

    key = (tuple(plan1.nch_lo), tuple(plan1.nch_hi),
           tuple(plan2.nch_lo), tuple(plan2.nch_hi))
    if key not in _CACHE:
        _CACHE[key] = _build(plan1, plan2)
    nc = _CACHE[key]

    in_maps = _host_inputs(
        inputs["x"], inputs["W1"], inputs["att_src1"], inputs["att_dst1"],
        inputs["b1"], inputs["W2"], inputs["att_src2"], inputs["att_dst2"],
        inputs["b2"], plan1, plan2)

    if trace:
        _install_ntff_hook()
    res = run_bass_kernel_spmd(nc, in_maps, core_ids=list(range(N_CORES)),
                               trace=trace)
    out = np.zeros((N_NODES, C2), np.float32)
    for c in range(N_CORES):
        o = res.results[c]["out"]
        mem = plan2.perm[c]
        valid = mem >= 0
        out[c * SHARD + mem[valid]] = o[valid]
    return out, res.exec_time_ns


def kernel(**inputs):
    out, _ = kernel_run(inputs)
    return out


# revision 4
# speedup vs baseline: 1.8313x; 1.8313x over previous
"""GAT 2-layer GNN kernel for 8 Trainium2 NeuronCores — v2.

Structure (vs the v1 baseline):
  - Nodes partitioned into 8 shards of 6250; per-core node features +
    attention scalars packed into 256-byte table rows; AllGather replicates
    the table (column-sliced: only the bytes the edge phase reads travel,
    and the AllGather is issued in 4 block-range pieces overlapped with
    table production).
  - Edges in ELL layout keyed by dst: dst slot = SBUF partition, per-dst
    edge list split into a lo run (table row < 32768) and a hi run, laid
    out per-block-contiguously [b0.lo|b0.hi|b1.lo|b1.hi|...] so each
    block's chunks are one contiguous range.
  - h[src]/a_src[src] fetched per-edge with dma_gather. Descriptor
    generation (~7.8ns/idx on the Q7) is the kernel bottleneck, so each
    group's gather is split into ~16-chunk segments spread over all 4
    SWDGE queues; queues 1-3 generate asynchronously, queue 0 inline on
    the Pool engine, giving ~4x parallel descriptor generation.
  - a_dst[dst] is never gathered: it stays on-chip in the per-core table
    SBUF copy (tab1_sb/tab2_sb) written during production.
  - Vector work is batched group-wide ([128, gch, H] ops) instead of
    per-run; the segment softmax-aggregate is identity-lhsT PSUM matmuls
    per chunk as before.
"""

import os
import sys

sys.path.insert(0, "/opt/trn_rl_repo")

import numpy as np
import ml_dtypes

import concourse.bacc as bacc
import concourse.mybir as mybir
from concourse import tile
from concourse.bass_utils import run_bass_kernel_spmd
from concourse.masks import make_identity

bf16 = ml_dtypes.bfloat16

N_NODES = 50000
F_IN = 512
H1 = 8
HID = 8
D1 = H1 * HID  # 64
C2 = 40
N_CORES = 8
SHARD = N_NODES // N_CORES  # 6250
BLK = 128
NB = (SHARD + BLK - 1) // BLK  # 49
SPLIT = 32768
HIBASE = N_NODES - SPLIT  # 17232
BASE1 = 8616  # middle gather base; BASES rows: [0,32768) [8616,41384) [17232,50000)
BASES = (0, BASE1, HIBASE)
SBG = 2  # blocks per super-group
SEG = int(os.environ.get("K_SEG", "16"))  # max chunks per gather segment
NEG_SLOPE = 0.2
TROW = 128  # table row: 128 bf16 = 256 bytes
NPIECE = 4

f32 = mybir.dt.float32
bfl = mybir.dt.bfloat16
i16 = mybir.dt.int16

# table col layouts (bf16 col units)
T1_AS = (64, 80)   # a_src1: 8 x f32
T1_AD = (80, 96)   # a_dst1: 8 x f32 (local only, not gathered/AG'd)
T1_AGC = 80        # AllGather cols 0:80 (160B)
T2_AS = (40, 42)   # a_src2: 1 x f32
T2_AD = (42, 44)   # a_dst2: local only
T2_AGC = 42        # AllGather cols 0:42 (84B)

_CACHE = {}


def _install_ntff_hook():
    """Provide antenv.axon_hooks if the image lacks it (NTFF profiling)."""
    try:
        from antenv.axon_hooks import get_axon_ntff_profile_hook  # noqa: F401
        return
    except ImportError:
        pass
    import contextlib
    import ctypes
    import types

    so_path = "/opt/axon/libaxon_pjrt.so"
    try:
        lib = ctypes.CDLL(so_path)
    except OSError:
        return
    if not hasattr(lib, "axon_start_nrt_profile"):
        return
    lib.axon_start_nrt_profile.argtypes = [ctypes.POINTER(ctypes.c_int64),
                                           ctypes.c_size_t]
    lib.axon_start_nrt_profile.restype = ctypes.c_int64
    lib.axon_stop_nrt_profile.argtypes = [ctypes.c_char_p]
    lib.axon_stop_nrt_profile.restype = ctypes.c_int64

    @contextlib.contextmanager
    def _hook(output_dir, device_ids):
        import jax
        jax.devices()
        if device_ids:
            ids = (ctypes.c_int64 * len(device_ids))(*device_ids)
            rc = lib.axon_start_nrt_profile(ids, len(device_ids))
        else:
            rc = lib.axon_start_nrt_profile(None, 0)
        if rc != 0:
            raise RuntimeError(f"axon_start_nrt_profile rc={rc}")
        try:
            yield
        finally:
            n = lib.axon_stop_nrt_profile(str(output_dir).encode())
            print(f"ntff profile: {n} file(s) written to {output_dir}")

    import antenv
    mod = types.ModuleType("antenv.axon_hooks")
    mod.get_axon_ntff_profile_hook = lambda: _hook
    mod.set_axon_ntff_profile_hook = lambda h: None
    sys.modules["antenv.axon_hooks"] = mod
    antenv.axon_hooks = mod


def _ceil(a, b):
    return (a + b - 1) // b


def _running_count(k):
    """pos[i] = number of j<i with k[j]==k[i]; k is sorted."""
    n = len(k)
    if n == 0:
        return np.zeros(0, np.int64)
    starts = np.r_[0, np.flatnonzero(np.diff(k)) + 1]
    run_id = np.zeros(n, np.int64)
    run_id[starts[1:]] = 1
    run_id = np.cumsum(run_id)
    return np.arange(n) - starts[run_id]


QMAP = [int(c) for c in os.environ.get("K_QMAP", "1230")]


class LayerPlan:
    pass


def _plan_layer(src_row, dst_node, perm, slot_of):
    """ELL plan with a 3-base flexible split: gather bases at rows 0, 8616,
    17232 (each covering 32768 rows). Every row is reachable from >=2 bases,
    so per-block run quotas K0/K1/K2 pack each block near its max-degree
    bound. perm/slot_of: the common slot permutation."""
    plan = LayerPlan()
    core = dst_node // SHARD
    local = dst_node - core * SHARD
    # classes: 0:[0,B1) run0 | 1:[B1,HIBASE) runs01 | 2:[HIBASE,SPLIT) any
    #          3:[SPLIT,B1+SPLIT) runs12 | 4:[B1+SPLIT,N) run2
    cls = np.digitize(src_row, [BASE1, HIBASE, SPLIT, BASE1 + SPLIT])

    cnt = np.zeros((5, N_CORES, SHARD), np.int64)
    for k in range(5):
        np.add.at(cnt[k], (core, local), (cls == k).astype(np.int64))
    n0, n01, n012, n12, n2 = cnt
    deg = cnt.sum(axis=0)

    order = perm[:, :SHARD]
    pad = np.zeros((N_CORES, NB * BLK), np.int64)
    def blockmax(x):
        p = pad.copy()
        p[:, :SHARD] = np.take_along_axis(x, order, 1)
        return p.reshape(N_CORES, NB, BLK).max(axis=(0, 2))
    M0 = np.maximum(blockmax(n0), 1)
    M01 = blockmax(n0 + n01)
    M12b = blockmax(n12 + n2)
    M2 = np.maximum(blockmax(n2), 1)
    D = blockmax(deg)
    T = np.maximum.reduce([D, M01 + M2, M0 + M12b, M0 + M2 + 1,
                           np.full(NB, 3, np.int64)])
    K0 = M0
    K2 = M2
    K1 = np.maximum.reduce([M01 - K0, M12b - K2, np.ones(NB, np.int64)])
    K1 = K1 + (T - (K0 + K1 + K2))
    Ks = np.stack([K0, K1, K2])          # [3, NB]
    assert (Ks > 0).all() and (Ks.sum(axis=0) == T).all()

    run_start = np.zeros((NB, 3), np.int64)   # global chunk offset per run
    groups = []
    goff = 0
    qload = {q: 0 for q in QMAP}
    for g in range(_ceil(NB, SBG)):
        blocks = list(range(g * SBG, min((g + 1) * SBG, NB)))
        ch = goff
        runs = []
        segruns = []
        for b in blocks:
            l0 = ch - goff
            for r in range(3):
                run_start[b, r] = ch
                segruns.append((ch - goff, int(Ks[r, b]), r))
                ch += int(Ks[r, b])
            runs.append((l0, int(T[b])))
        gch = ch - goff
        raw = []
        for (r0, rn, base) in segruns:
            c = r0
            while c < r0 + rn:
                n = min(SEG, r0 + rn - c)
                raw.append((c, n, base))
                c += n
        segs = []
        for (c, n, base) in sorted(raw, key=lambda x: -x[1]):
            q = min(QMAP, key=lambda qq: qload[qq])
            qload[q] += n
            segs.append((c, n, base, q))
        segs = [x for x in segs if x[3] != 0] + [x for x in segs if x[3] == 0]
        groups.append({"blocks": blocks, "goff": goff, "gch": gch,
                       "runs": runs, "segs": segs})
        goff = ch
    total_ch = goff

    idx_streams, mask_streams = [], []
    for c in range(N_CORES):
        sel = core == c
        e_row = src_row[sel]
        e_loc = local[sel]
        e_cls = cls[sel]
        e_slot = slot_of[c, e_loc]
        e_blk = e_slot // BLK

        # per-slot class counts and greedy run-fill quotas for this core
        cn = cnt[:, c, :]                      # [5, SHARD]
        K0s = K0[slot_of[c] // BLK]
        K1s = K1[slot_of[c] // BLK]
        t0_01 = np.minimum(cn[1], np.maximum(0, K0s - cn[0]))
        t0_012 = np.minimum(cn[2], np.maximum(0, K0s - cn[0] - t0_01))
        t1_01 = cn[1] - t0_01
        t1_12 = np.minimum(cn[3], np.maximum(0, K1s - t1_01))
        t1_012 = np.minimum(cn[2] - t0_012,
                            np.maximum(0, K1s - t1_01 - t1_12))
        rem12 = cn[3] - t1_12
        rem012 = cn[2] - t0_012 - t1_012
        assert (rem12 + rem012 + cn[4] <= K2[slot_of[c] // BLK] + 0).all()

        o = np.lexsort((e_cls, e_slot))
        r = _running_count(e_slot[o] * 8 + e_cls[o])  # rank in (slot, class)
        sl = e_loc[o]
        ecl = e_cls[o]
        run = np.zeros(len(o), np.int64)
        pos = np.zeros(len(o), np.int64)
        m = ecl == 0
        run[m], pos[m] = 0, r[m]
        m = ecl == 1
        in0 = r[m] < t0_01[sl[m]]
        run[m] = np.where(in0, 0, 1)
        pos[m] = np.where(in0, cn[0][sl[m]] + r[m], r[m] - t0_01[sl[m]])
        m = ecl == 2
        rr = r[m]
        in0 = rr < t0_012[sl[m]]
        in1 = (~in0) & (rr - t0_012[sl[m]] < t1_012[sl[m]])
        run[m] = np.where(in0, 0, np.where(in1, 1, 2))
        pos[m] = np.where(
            in0, cn[0][sl[m]] + t0_01[sl[m]] + rr,
            np.where(in1,
                     t1_01[sl[m]] + t1_12[sl[m]] + (rr - t0_012[sl[m]]),
                     rem12[sl[m]] + (rr - t0_012[sl[m]] - t1_012[sl[m]])))
        m = ecl == 3
        in1 = r[m] < t1_12[sl[m]]
        run[m] = np.where(in1, 1, 2)
        pos[m] = np.where(in1, t1_01[sl[m]] + r[m], r[m] - t1_12[sl[m]])
        m = ecl == 4
        run[m] = 2
        pos[m] = rem12[sl[m]] + rem012[sl[m]] + r[m]

        blk_o = e_blk[o]
        chunk = run_start[blk_o, run] + pos
        slots = chunk * BLK + (e_slot[o] % BLK)
        rows_o = e_row[o]
        basev = np.array(BASES, np.int64)[run]
        assert (rows_o - basev >= 0).all() and (rows_o - basev < SPLIT).all()

        idx = np.zeros(total_ch * BLK, np.int16)
        mask = np.zeros(total_ch * BLK, np.float32)
        idx[slots] = (rows_o - basev).astype(np.int16)
        mask[slots] = 1.0

        idx_w = np.tile(idx.reshape(total_ch * 8, 16).T, (8, 1)).copy()
        mask_w = mask.reshape(total_ch, BLK).T.astype(bf16).copy()
        idx_streams.append(idx_w)
        mask_streams.append(mask_w)

    plan.tot = T
    plan.groups = groups
    plan.total_ch = total_ch
    plan.idx_streams = idx_streams
    plan.mask_streams = mask_streams
    plan.perm = perm
    return plan


def _prep(edge_index):
    """One common slot permutation; x is host-permuted into slot order, so
    both layers share one table row space and ONE plan (identical streams)."""
    src = np.asarray(edge_index[0], dtype=np.int64)
    dst = np.asarray(edge_index[1], dtype=np.int64)
    # self-loops are handled analytically on-chip, not in the edge streams

    row_pm = _row_of_piece_major()
    s_core = src // SHARD
    s_local = src - s_core * SHARD
    core = dst // SHARD
    local = dst - core * SHARD
    deg = np.zeros((N_CORES, SHARD), np.int64)
    np.add.at(deg, (core, local), 1)

    order = np.argsort(-deg, axis=1, kind="stable")
    for _ in range(2):
        slot_of = np.zeros((N_CORES, SHARD), np.int64)
        for c in range(N_CORES):
            slot_of[c, order[c]] = np.arange(SHARD)
        srow = row_pm[s_core, slot_of[s_core, s_local]]
        klo_f = np.zeros((N_CORES, SHARD), np.int64)
        np.add.at(klo_f, (core, local), (srow < BASE1).astype(np.int64))
        order = np.lexsort((-klo_f, -deg), axis=-1)

    perm = np.full((N_CORES, NB * BLK), -1, np.int64)
    slot_of = np.zeros((N_CORES, SHARD), np.int64)
    for c in range(N_CORES):
        perm[c, :SHARD] = order[c]
        slot_of[c, order[c]] = np.arange(SHARD)
    src_row = row_pm[s_core, slot_of[s_core, s_local]]
    plan = _plan_layer(src_row, dst, perm, slot_of)
    return plan, plan


def _piece_bounds():
    """NPIECE block ranges aligned to group (SBG) boundaries."""
    ngroups = _ceil(NB, SBG)
    gb = [round(i * ngroups / NPIECE) for i in range(NPIECE + 1)]
    return [(gb[i], gb[i + 1], min(gb[i] * SBG, NB), min(gb[i + 1] * SBG, NB))
            for i in range(NPIECE)]


def _piece_rows():
    """Local-row ranges per piece and piece-major t_full bases (in rows)."""
    pr = [min(be * BLK, SHARD) for (_, _, _, be) in _piece_bounds()]
    pr = [0] + pr
    base = [N_CORES * r for r in pr]
    return pr, base


def _row_of_piece_major():
    """row_of[c, local] = piece-major t_full row of core c's local row."""
    pr, base = _piece_rows()
    row_of = np.zeros((N_CORES, SHARD), np.int64)
    for p in range(NPIECE):
        r0, r1 = pr[p], pr[p + 1]
        ln = r1 - r0
        for c in range(N_CORES):
            row_of[c, r0:r1] = base[p] + c * ln + np.arange(ln)
    return row_of


def _dma_blocks_out(nc, shard_dram, tab_sb, bs, be):
    """DMA tab_sb[:, bs:be, :] to shard_dram rows [bs*BLK, min(be*BLK, SHARD))."""
    r0 = bs * BLK
    r1 = min(be * BLK, SHARD)
    full = (r1 - r0) // BLK  # full blocks
    if full > 0:
        nc.scalar.dma_start(
            out=shard_dram[r0:r0 + full * BLK, :].rearrange(
                "(b p) c -> p b c", p=BLK, b=full),
            in_=tab_sb[:, bs:bs + full, :])
    rem = (r1 - r0) - full * BLK
    if rem > 0:
        nc.scalar.dma_start(out=shard_dram[r0 + full * BLK:r1, :],
                          in_=tab_sb[0:rem, bs + full, :])


AG_MODE = int(os.environ.get("K_AG_MODE", "3"))
GQMAP = [int(c) for c in os.environ.get("K_QMAP", "1230")]


def _ag_piece(nc, shard_dram, full_dram, bs, be, ncols, rg):
    """AllGather rows [bs*BLK, min(be*BLK, SHARD)) x cols [0:ncols)."""
    if AG_MODE == 0:
        return
    r0 = bs * BLK
    r1 = min(be * BLK, SHARD)
    if AG_MODE == 3:
        pr, base = _piece_rows()
        p = pr.index(r0)
        assert pr[p + 1] == r1
        nc.gpsimd.collective_compute(
            "AllGather", mybir.AluOpType.bypass, replica_groups=rg,
            ins=[shard_dram[r0:r1, :].opt()],
            outs=[full_dram[base[p]:base[p + 1], :].opt()])
        return
    full_v = full_dram.ap().rearrange("(r n) c -> r n c", r=N_CORES)
    if AG_MODE == 2:
        ncols = TROW
    nc.gpsimd.collective_compute(
        "AllGather", mybir.AluOpType.bypass, replica_groups=rg,
        ins=[shard_dram[r0:r1, 0:ncols]],
        outs=[full_v[:, r0:r1, 0:ncols]])


def _ag_whole(nc, shard_dram, full_dram, rg):
    if AG_MODE != 0:
        return
    nc.gpsimd.collective_compute(
        "AllGather", mybir.AluOpType.bypass, replica_groups=rg,
        ins=[shard_dram.ap().opt()], outs=[full_dram.ap().opt()])


def _build(plan1, plan2):
    nc = bacc.Bacc("TRN2", target_bir_lowering=False, debug=False,
                   num_devices=N_CORES, num_swdge_queues=4)

    NPADROWS = NB * BLK  # 6272
    xT_ext = nc.declare_dram_parameter("xT", [F_IN, NPADROWS], bfl, isOutput=False)
    w1_ext = nc.declare_dram_parameter("w1r", [128, 4 * D1], bfl, isOutput=False)
    w2_ext = nc.declare_dram_parameter("w2", [D1, C2], bfl, isOutput=False)
    a1s_ext = nc.declare_dram_parameter("a1srep", [128, D1], f32, isOutput=False)
    a1d_ext = nc.declare_dram_parameter("a1drep", [128, D1], f32, isOutput=False)
    a2s_ext = nc.declare_dram_parameter("a2srep", [128, C2], f32, isOutput=False)
    a2d_ext = nc.declare_dram_parameter("a2drep", [128, C2], f32, isOutput=False)
    b1_ext = nc.declare_dram_parameter("b1rep", [128, D1], f32, isOutput=False)
    b2_ext = nc.declare_dram_parameter("b2rep", [128, C2], f32, isOutput=False)
    idx1_ext = nc.declare_dram_parameter("idx1", [128, plan1.total_ch * 8], i16,
                                         isOutput=False)
    msk1_ext = nc.declare_dram_parameter("msk1", [128, plan1.total_ch], bfl,
                                         isOutput=False)
    out_ext = nc.declare_dram_parameter("out", [NB * BLK, C2], f32, isOutput=True)

    t1_shard = nc.dram_tensor("t1_shard", [SHARD, TROW], bfl)
    t1_full = nc.dram_tensor("t1_full", [N_NODES, TROW], bfl, addr_space="Shared")
    t2_shard = nc.dram_tensor("t2_shard", [SHARD, TROW], bfl)
    t2_full = nc.dram_tensor("t2_full", [N_NODES, TROW], bfl, addr_space="Shared")

    rg = [list(range(N_CORES))]
    pieces = _piece_bounds()

    with tile.TileContext(nc) as tc:
        with tc.tile_pool(name="const", bufs=1) as cpool:
            ident = cpool.tile([128, 128], bfl)
            make_identity(nc, ident[:, :])
            a1s_t = cpool.tile([128, D1], f32)
            nc.sync.dma_start(out=a1s_t[:, :], in_=a1s_ext[:, :])
            a1d_t = cpool.tile([128, D1], f32)
            nc.sync.dma_start(out=a1d_t[:, :], in_=a1d_ext[:, :])
            a2s_t = cpool.tile([128, C2], f32)
            nc.sync.dma_start(out=a2s_t[:, :], in_=a2s_ext[:, :])
            a2d_t = cpool.tile([128, C2], f32)
            nc.sync.dma_start(out=a2d_t[:, :], in_=a2d_ext[:, :])
            b1_t = cpool.tile([128, D1], f32)
            nc.sync.dma_start(out=b1_t[:, :], in_=b1_ext[:, :])
            b2_t = cpool.tile([128, C2], f32)
            nc.sync.dma_start(out=b2_t[:, :], in_=b2_ext[:, :])
            w2_t = cpool.tile([D1, C2], bfl)
            nc.sync.dma_start(out=w2_t[:, :], in_=w2_ext[:, :])
            tab1_sb = cpool.tile([128, NB, TROW], bfl)
            tab2_sb = cpool.tile([128, NB, TROW], bfl)
            nc.vector.memset(tab1_sb[:, :, :], 0.0)
            nc.vector.memset(tab2_sb[:, :, :], 0.0)
            idx_all = cpool.tile([128, plan1.total_ch * 8], i16)
            nc.sync.dma_start(out=idx_all[:, :], in_=idx1_ext[:, :])
            msk_all = cpool.tile([128, plan1.total_ch], bfl)
            nc.sync.dma_start(out=msk_all[:, :], in_=msk1_ext[:, :])


            # ---- Phase A: h1 = x @ W1 + attention scalars, piecewise AG1
            with tc.tile_pool(name="phA", bufs=2) as apool, \
                 tc.tile_pool(name="phA_ps", bufs=4, space="PSUM") as apsum:
                w1_t = apool.tile([128, 4, D1], bfl, tag="w1")
                nc.sync.dma_start(out=w1_t[:, :, :], in_=w1_ext[:, :])
                xk = []
                for k in range(4):
                    xt = apool.tile([128, NPADROWS], bfl, tag=f"xk{k}")
                    nc.sync.dma_start(out=xt[:, :],
                                      in_=xT_ext[k * 128:(k + 1) * 128, :])
                    xk.append(xt)
                for (g0, g1, bs, be) in pieces:
                    for b in range(bs, be):
                        hps = apsum.tile([128, D1], f32, tag="hps")
                        for k in range(4):
                            nc.tensor.matmul(
                                hps[:, :],
                                lhsT=xk[k][:, b * BLK:(b + 1) * BLK],
                                rhs=w1_t[:, k, :], start=(k == 0), stop=(k == 3))
                        nc.scalar.activation(
                            out=tab1_sb[:, b, 0:D1], in_=hps[:, :],
                            func=mybir.ActivationFunctionType.Copy)
                        for a_t, sl in ((a1s_t, T1_AS), (a1d_t, T1_AD)):
                            tmp = apool.tile([128, D1], f32, tag="atmp")
                            nc.vector.tensor_tensor(
                                out=tmp[:, :], in0=hps[:, :], in1=a_t[:, :],
                                op=mybir.AluOpType.mult)
                            nc.vector.tensor_reduce(
                                out=tab1_sb[:, b, sl[0]:sl[1]].bitcast(f32),
                                in_=tmp[:, :].rearrange(
                                    "p (h c) -> p h c", h=H1, c=HID),
                                axis=mybir.AxisListType.X,
                                op=mybir.AluOpType.add)
                    _dma_blocks_out(nc, t1_shard, tab1_sb, bs, be)
                    _ag_piece(nc, t1_shard, t1_full, bs, be, T1_AGC, rg)

            _ag_whole(nc, t1_shard, t1_full, rg)

            _edge_phase(nc, tc, layer=1, table_full=t1_full,
                        idx_all=idx_all, msk_all=msk_all, ident=ident,
                        plan=plan1, pieces=pieces, bias=b1_t, w2_t=w2_t,
                        a2s_t=a2s_t, a2d_t=a2d_t, tab_sb=tab1_sb,
                        tab_out=tab2_sb, t_shard=t2_shard, t_full=t2_full,
                        rg=rg, out_ext=None, b2_t=None)

            _ag_whole(nc, t2_shard, t2_full, rg)

            _edge_phase(nc, tc, layer=2, table_full=t2_full,
                        idx_all=idx_all, msk_all=msk_all, ident=ident,
                        plan=plan2, pieces=pieces, bias=None, w2_t=None,
                        a2s_t=None, a2d_t=None, tab_sb=tab2_sb,
                        tab_out=None, t_shard=None, t_full=None,
                        rg=rg, out_ext=out_ext, b2_t=b2_t)

    nc.compile()
    return nc


def _edge_phase(nc, tc, layer, table_full, idx_all, msk_all, ident,
                plan, pieces, bias, w2_t, a2s_t, a2d_t, tab_sb,
                tab_out, t_shard, t_full, rg, out_ext, b2_t):
    if layer == 1:
        NH, CH, CC = H1, HID, D1
        asrc_sl, adst_sl = T1_AS, T1_AD
    else:
        NH, CH, CC = 1, C2, C2
        asrc_sl, adst_sl = T2_AS, T2_AD
    NCOL = CC + NH
    gmax = max(g["gch"] for g in plan.groups)

    with tc.tile_pool(name=f"self{layer}", bufs=1) as spool, \
         tc.tile_pool(name=f"e{layer}", bufs=2) as pool, \
         tc.tile_pool(name=f"e{layer}_ps", bufs=2, space="PSUM") as psum, \
         tc.tile_pool(name=f"e{layer}_ps2", bufs=2, space="PSUM") as psum2:
        # analytic self-loop term: w = exp(leakyrelu(a_src[d]+a_dst[d])),
        # num += w*h[d], den += w  (self-loops excluded from edge streams)
        es = spool.tile([128, NB, NH], f32)
        nc.vector.tensor_tensor(
            out=es[:, :, :],
            in0=tab_sb[:, :, asrc_sl[0]:asrc_sl[1]].bitcast(f32),
            in1=tab_sb[:, :, adst_sl[0]:adst_sl[1]].bitcast(f32),
            op=mybir.AluOpType.add)
        lrs = spool.tile([128, NB, NH], f32)
        nc.vector.scalar_tensor_tensor(
            out=lrs[:, :, :], in0=es[:, :, :], scalar=NEG_SLOPE,
            in1=es[:, :, :], op0=mybir.AluOpType.mult,
            op1=mybir.AluOpType.max)
        ws = spool.tile([128, NB, NH], f32)
        nc.scalar.activation(out=ws[:, :, :], in_=lrs[:, :, :],
                             func=mybir.ActivationFunctionType.Exp)
        sn = spool.tile([128, NB, CC], f32)
        nc.vector.tensor_tensor(
            out=sn[:, :, :].rearrange("p b (h c) -> p b h c", h=NH, c=CH),
            in0=tab_sb[:, :, 0:CC].rearrange("p b (h c) -> p b h c",
                                             h=NH, c=CH),
            in1=ws[:, :, :, None].to_broadcast([128, NB, NH, CH]),
            op=mybir.AluOpType.mult)
        piece_of_group = {}
        for pi, (g0, g1, bs, be) in enumerate(pieces):
            for g in range(g0, g1):
                piece_of_group[g] = pi if g == g1 - 1 else None

        for gi, grp in enumerate(plan.groups):
            goff, gch = grp["goff"], grp["gch"]
            mskg = msk_all[:, goff:goff + gch]
            g_t = pool.tile([128, gmax, TROW], bfl, tag="gath")
            for (c0, n, base, q) in grp["segs"]:
                in_ap = table_full[BASES[base]:BASES[base] + SPLIT, :]
                nc.gpsimd.dma_gather(
                    out_ap=g_t[:, c0:c0 + n, :], in_ap=in_ap,
                    idxs_ap=idx_all[:, (goff + c0) * 8:(goff + c0 + n) * 8],
                    num_idxs=n * BLK, num_idxs_reg=n * BLK,
                    elem_size=TROW, single_packet=False, queue_num=q)

            # group-wide attention pipeline
            e_t = pool.tile([128, gmax, NH], f32, tag="elog")
            for b, (l0, rn) in zip(grp["blocks"], grp["runs"]):
                nc.vector.tensor_tensor(
                    out=e_t[:, l0:l0 + rn, :],
                    in0=g_t[:, l0:l0 + rn,
                            asrc_sl[0]:asrc_sl[1]].bitcast(f32),
                    in1=tab_sb[:, b, adst_sl[0]:adst_sl[1]].bitcast(f32)
                        [:, None, :].to_broadcast([128, rn, NH]),
                    op=mybir.AluOpType.add)
            lr_t = pool.tile([128, gmax, NH], f32, tag="lrt")
            nc.vector.scalar_tensor_tensor(
                out=lr_t[:, 0:gch, :], in0=e_t[:, 0:gch, :], scalar=NEG_SLOPE,
                in1=e_t[:, 0:gch, :], op0=mybir.AluOpType.mult,
                op1=mybir.AluOpType.max)
            exf = pool.tile([128, gmax, NH], f32, tag="exf")
            nc.scalar.activation(out=exf[:, 0:gch, :], in_=lr_t[:, 0:gch, :],
                                 func=mybir.ActivationFunctionType.Exp)
            r_t = pool.tile([128, gmax, NCOL], bfl, tag="rmat")
            nc.vector.tensor_tensor(
                out=r_t[:, 0:gch, CC:NCOL], in0=exf[:, 0:gch, :],
                in1=mskg[:, :, None].to_broadcast([128, gch, NH]),
                op=mybir.AluOpType.mult)
            nc.vector.tensor_tensor(
                out=r_t[:, 0:gch, 0:CC].rearrange("p g (h c) -> p g h c",
                                                  h=NH, c=CH),
                in0=g_t[:, 0:gch, 0:CC].rearrange("p g (h c) -> p g h c",
                                                  h=NH, c=CH),
                in1=r_t[:, 0:gch, CC:NCOL, None].to_broadcast(
                    [128, gch, NH, CH]),
                op=mybir.AluOpType.mult)

            for b, (l0, rn) in zip(grp["blocks"], grp["runs"]):
                ps = psum.tile([128, NCOL], f32, tag="agg")
                for j in range(rn):
                    nc.tensor.matmul(ps[:, :], lhsT=ident[:, :],
                                     rhs=r_t[:, l0 + j, :], start=(j == 0),
                                     stop=(j == rn - 1))
                den = pool.tile([128, NH], f32, tag="den")
                nc.vector.scalar_tensor_tensor(
                    out=den[:, :], in0=ps[:, CC:NCOL], scalar=1e-16,
                    in1=ws[:, b, :], op0=mybir.AluOpType.add,
                    op1=mybir.AluOpType.add)
                recip = pool.tile([128, NH], f32, tag="recip")
                nc.vector.reciprocal(out=recip[:, :], in_=den[:, :])
                onum = pool.tile([128, CC], f32, tag="onum")
                nc.vector.tensor_tensor(out=onum[:, :], in0=ps[:, 0:CC],
                                        in1=sn[:, b, :],
                                        op=mybir.AluOpType.add)
                o_t = pool.tile([128, CC], f32, tag="outb")
                nc.vector.tensor_tensor(
                    out=o_t[:, :].rearrange("p (h c) -> p h c", h=NH, c=CH),
                    in0=onum[:, :].rearrange("p (h c) -> p h c", h=NH, c=CH),
                    in1=recip[:, :, None].to_broadcast([128, NH, CH]),
                    op=mybir.AluOpType.mult)

                if layer == 1:
                    obt = pool.tile([128, CC], f32, tag="outbt")
                    nc.vector.tensor_tensor(out=obt[:, :], in0=o_t[:, :],
                                            in1=bias[:, :],
                                            op=mybir.AluOpType.add)
                    ob = pool.tile([128, CC], bfl, tag="outbf")
                    nc.vector.tensor_scalar(out=ob[:, :], in0=obt[:, :],
                                            scalar1=0.0, scalar2=None,
                                            op0=mybir.AluOpType.max)
                    tps = psum2.tile([D1, 128], bfl, tag="tp")
                    nc.tensor.transpose(tps[:, :], ob[:, :], ident[:, :])
                    h1T = pool.tile([D1, 128], bfl, tag="h1T")
                    nc.vector.tensor_copy(out=h1T[:, :], in_=tps[:, :])
                    h2ps = psum2.tile([128, C2], f32, tag="h2")
                    nc.tensor.matmul(h2ps[:, :], lhsT=h1T[:, :], rhs=w2_t[:, :],
                                     start=True, stop=True)
                    nc.scalar.activation(out=tab_out[:, b, 0:C2],
                                         in_=h2ps[:, :],
                                         func=mybir.ActivationFunctionType.Copy)
                    for a_t, sl in ((a2s_t, T2_AS), (a2d_t, T2_AD)):
                        t2a = pool.tile([128, C2], f32, tag="t2a")
                        nc.vector.tensor_tensor(out=t2a[:, :], in0=h2ps[:, :],
                                                in1=a_t[:, :],
                                                op=mybir.AluOpType.mult)
                        nc.vector.tensor_reduce(
                            out=tab_out[:, b, sl[0]:sl[1]].bitcast(f32),
                            in_=t2a[:, :], axis=mybir.AxisListType.X,
                            op=mybir.AluOpType.add)
                else:
                    lg = pool.tile([128, C2], f32, tag="logits")
                    nc.vector.tensor_tensor(out=lg[:, :], in0=o_t[:, :],
                                            in1=b2_t[:, :],
                                            op=mybir.AluOpType.add)
                    negm = pool.tile([128, 1], f32, tag="negm")
                    nc.vector.tensor_reduce(out=negm[:, :], in_=lg[:, :],
                                            axis=mybir.AxisListType.X,
                                            op=mybir.AluOpType.max, negate=True)
                    ex = pool.tile([128, C2], f32, tag="sfex")
                    ssum = pool.tile([128, 1], f32, tag="ssum")
                    nc.scalar.activation(out=ex[:, :], in_=lg[:, :],
                                         func=mybir.ActivationFunctionType.Exp,
                                         bias=negm[:, :], accum_out=ssum[:, :])
                    lse = pool.tile([128, 1], f32, tag="lse")
                    nc.scalar.activation(out=lse[:, :], in_=ssum[:, :],
                                         func=mybir.ActivationFunctionType.Ln)
                    res = pool.tile([128, C2], f32, tag="res")
                    nc.vector.scalar_tensor_tensor(
                        out=res[:, :], in0=lg[:, :], scalar=negm[:, :],
                        in1=lse[:, :].to_broadcast([128, C2]),
                        op0=mybir.AluOpType.add, op1=mybir.AluOpType.subtract)
                    nc.scalar.dma_start(out=out_ext[b * BLK:(b + 1) * BLK, :],
                                      in_=res[:, :])

            if layer == 1:
                pi = piece_of_group.get(gi)
                if pi is not None:
                    g0, g1, bs, be = pieces[pi]
                    _dma_blocks_out(nc, t_shard, tab_out, bs, be)
                    _ag_piece(nc, t_shard, t_full, bs, be, T2_AGC, rg)


def _host_inputs(x, W1, att_src1, att_dst1, b1, W2, att_src2, att_dst2, b2,
                 plan1, plan2):
    NPADROWS = NB * BLK
    w1r = np.ascontiguousarray(
        np.asarray(W1, np.float32).reshape(4, 128, D1).transpose(1, 0, 2)
    ).reshape(128, 4 * D1).astype(bf16)
    rep = lambda v, n: np.tile(np.asarray(v, np.float32).reshape(1, n),
                               (128, 1)).astype(np.float32)
    x32 = np.asarray(x, np.float32)

    in_maps = []
    for c in range(N_CORES):
        xs = x32[c * SHARD:(c + 1) * SHARD][plan1.perm[c, :SHARD]]
        xT = np.zeros((F_IN, NPADROWS), bf16)
        xT[:, :SHARD] = xs.T.astype(bf16)
        in_maps.append({
            "xT": xT,
            "w1r": w1r,
            "w2": np.asarray(W2, np.float32).astype(bf16),
            "a1srep": rep(att_src1, D1),
            "a1drep": rep(att_dst1, D1),
            "a2srep": rep(att_src2, C2),
            "a2drep": rep(att_dst2, C2),
            "b1rep": rep(b1, D1),
            "b2rep": rep(b2, C2),
            "idx1": plan1.idx_streams[c],
            "msk1": plan1.mask_streams[c],
        })
    return in_maps


def kernel_run(inputs, trace=False):
    edge_index = inputs["edge_index"]
    plan1, plan2 = _prep(edge_index)

    key = tuple(plan1.tot)
    if key not in _CACHE:
        _CACHE[key] = _build(plan1, plan2)
    nc = _CACHE[key]

    in_maps = _host_inputs(
        inputs["x"], inputs["W1"], inputs["att_src1"], inputs["att_dst1"],
        inputs["b1"], inputs["W2"], inputs["att_src2"], inputs["att_dst2"],
        inputs["b2"], plan1, plan2)

    if trace:
        _install_ntff_hook()
    res = run_bass_kernel_spmd(nc, in_maps, core_ids=list(range(N_CORES)),
                               trace=trace)
    out = np.zeros((N_NODES, C2), np.float32)
    for c in range(N_CORES):
        o = res.results[c]["out"]
        mem = plan2.perm[c]
        valid = mem >= 0
        out[c * SHARD + mem[valid]] = o[valid]
    return out, res.exec_time_ns


def kernel(**inputs):
    out, _ = kernel_run(inputs)
    return out


# revision 5
# speedup vs baseline: 1.8453x; 1.0076x over previous
"""GAT 2-layer GNN kernel for 8 Trainium2 NeuronCores — v2.

Structure (vs the v1 baseline):
  - Nodes partitioned into 8 shards of 6250; per-core node features +
    attention scalars packed into 256-byte table rows; AllGather replicates
    the table (column-sliced: only the bytes the edge phase reads travel,
    and the AllGather is issued in 4 block-range pieces overlapped with
    table production).
  - Edges in ELL layout keyed by dst: dst slot = SBUF partition, per-dst
    edge list split into a lo run (table row < 32768) and a hi run, laid
    out per-block-contiguously [b0.lo|b0.hi|b1.lo|b1.hi|...] so each
    block's chunks are one contiguous range.
  - h[src]/a_src[src] fetched per-edge with dma_gather. Descriptor
    generation (~7.8ns/idx on the Q7) is the kernel bottleneck, so each
    group's gather is split into ~16-chunk segments spread over all 4
    SWDGE queues; queues 1-3 generate asynchronously, queue 0 inline on
    the Pool engine, giving ~4x parallel descriptor generation.
  - a_dst[dst] is never gathered: it stays on-chip in the per-core table
    SBUF copy (tab1_sb/tab2_sb) written during production.
  - Vector work is batched group-wide ([128, gch, H] ops) instead of
    per-run; the segment softmax-aggregate is identity-lhsT PSUM matmuls
    per chunk as before.
"""

import os
import sys

sys.path.insert(0, "/opt/trn_rl_repo")

import numpy as np
import ml_dtypes

import concourse.bacc as bacc
import concourse.mybir as mybir
from concourse import tile
from concourse.bass_utils import run_bass_kernel_spmd
from concourse.masks import make_identity

bf16 = ml_dtypes.bfloat16

N_NODES = 50000
F_IN = 512
H1 = 8
HID = 8
D1 = H1 * HID  # 64
C2 = 40
N_CORES = 8
SHARD = N_NODES // N_CORES  # 6250
BLK = 128
NB = (SHARD + BLK - 1) // BLK  # 49
SPLIT = 32768
HIBASE = N_NODES - SPLIT  # 17232
BASE1 = 8616  # middle gather base; BASES rows: [0,32768) [8616,41384) [17232,50000)
BASES = (0, BASE1, HIBASE)
SBG = 2  # blocks per super-group
SEG = int(os.environ.get("K_SEG", "24"))  # max chunks per gather segment
NEG_SLOPE = 0.2
TROW = 128  # table row: 128 bf16 = 256 bytes
NPIECE = 4

f32 = mybir.dt.float32
bfl = mybir.dt.bfloat16
i16 = mybir.dt.int16

# table col layouts (bf16 col units)
T1_AS = (64, 80)   # a_src1: 8 x f32
T1_AD = (80, 96)   # a_dst1: 8 x f32 (local only, not gathered/AG'd)
T1_AGC = 80        # AllGather cols 0:80 (160B)
T2_AS = (40, 42)   # a_src2: 1 x f32
T2_AD = (42, 44)   # a_dst2: local only
T2_AGC = 42        # AllGather cols 0:42 (84B)

_CACHE = {}


def _install_ntff_hook():
    """Provide antenv.axon_hooks if the image lacks it (NTFF profiling)."""
    try:
        from antenv.axon_hooks import get_axon_ntff_profile_hook  # noqa: F401
        return
    except ImportError:
        pass
    import contextlib
    import ctypes
    import types

    so_path = "/opt/axon/libaxon_pjrt.so"
    try:
        lib = ctypes.CDLL(so_path)
    except OSError:
        return
    if not hasattr(lib, "axon_start_nrt_profile"):
        return
    lib.axon_start_nrt_profile.argtypes = [ctypes.POINTER(ctypes.c_int64),
                                           ctypes.c_size_t]
    lib.axon_start_nrt_profile.restype = ctypes.c_int64
    lib.axon_stop_nrt_profile.argtypes = [ctypes.c_char_p]
    lib.axon_stop_nrt_profile.restype = ctypes.c_int64

    @contextlib.contextmanager
    def _hook(output_dir, device_ids):
        import jax
        jax.devices()
        if device_ids:
            ids = (ctypes.c_int64 * len(device_ids))(*device_ids)
            rc = lib.axon_start_nrt_profile(ids, len(device_ids))
        else:
            rc = lib.axon_start_nrt_profile(None, 0)
        if rc != 0:
            raise RuntimeError(f"axon_start_nrt_profile rc={rc}")
        try:
            yield
        finally:
            n = lib.axon_stop_nrt_profile(str(output_dir).encode())
            print(f"ntff profile: {n} file(s) written to {output_dir}")

    import antenv
    mod = types.ModuleType("antenv.axon_hooks")
    mod.get_axon_ntff_profile_hook = lambda: _hook
    mod.set_axon_ntff_profile_hook = lambda h: None
    sys.modules["antenv.axon_hooks"] = mod
    antenv.axon_hooks = mod


def _ceil(a, b):
    return (a + b - 1) // b


def _running_count(k):
    """pos[i] = number of j<i with k[j]==k[i]; k is sorted."""
    n = len(k)
    if n == 0:
        return np.zeros(0, np.int64)
    starts = np.r_[0, np.flatnonzero(np.diff(k)) + 1]
    run_id = np.zeros(n, np.int64)
    run_id[starts[1:]] = 1
    run_id = np.cumsum(run_id)
    return np.arange(n) - starts[run_id]


QMAP = [int(c) for c in os.environ.get("K_QMAP", "1230")]


class LayerPlan:
    pass


def _plan_layer(src_row, dst_node, perm, slot_of):
    """ELL plan with a 3-base flexible split: gather bases at rows 0, 8616,
    17232 (each covering 32768 rows). Every row is reachable from >=2 bases,
    so per-block run quotas K0/K1/K2 pack each block near its max-degree
    bound. perm/slot_of: the common slot permutation."""
    plan = LayerPlan()
    core = dst_node // SHARD
    local = dst_node - core * SHARD
    # classes: 0:[0,B1) run0 | 1:[B1,HIBASE) runs01 | 2:[HIBASE,SPLIT) any
    #          3:[SPLIT,B1+SPLIT) runs12 | 4:[B1+SPLIT,N) run2
    cls = np.digitize(src_row, [BASE1, HIBASE, SPLIT, BASE1 + SPLIT])

    cnt = np.zeros((5, N_CORES, SHARD), np.int64)
    for k in range(5):
        np.add.at(cnt[k], (core, local), (cls == k).astype(np.int64))
    n0, n01, n012, n12, n2 = cnt
    deg = cnt.sum(axis=0)

    order = perm[:, :SHARD]
    pad = np.zeros((N_CORES, NB * BLK), np.int64)
    def blockmax(x):
        p = pad.copy()
        p[:, :SHARD] = np.take_along_axis(x, order, 1)
        return p.reshape(N_CORES, NB, BLK).max(axis=(0, 2))
    M0 = np.maximum(blockmax(n0), 1)
    M01 = blockmax(n0 + n01)
    M12b = blockmax(n12 + n2)
    M2 = np.maximum(blockmax(n2), 1)
    D = blockmax(deg)
    T = np.maximum.reduce([D, M01 + M2, M0 + M12b, M0 + M2 + 1,
                           np.full(NB, 3, np.int64)])
    K0 = M0
    K2 = M2
    K1 = np.maximum.reduce([M01 - K0, M12b - K2, np.ones(NB, np.int64)])
    K1 = K1 + (T - (K0 + K1 + K2))
    Ks = np.stack([K0, K1, K2])          # [3, NB]
    assert (Ks > 0).all() and (Ks.sum(axis=0) == T).all()

    run_start = np.zeros((NB, 3), np.int64)   # global chunk offset per run
    groups = []
    goff = 0
    qload = {q: 0 for q in QMAP}
    for g in range(_ceil(NB, SBG)):
        blocks = list(range(g * SBG, min((g + 1) * SBG, NB)))
        ch = goff
        runs = []
        segruns = []
        for b in blocks:
            l0 = ch - goff
            for r in range(3):
                run_start[b, r] = ch
                segruns.append((ch - goff, int(Ks[r, b]), r))
                ch += int(Ks[r, b])
            runs.append((l0, int(T[b])))
        gch = ch - goff
        raw = []
        for (r0, rn, base) in segruns:
            c = r0
            while c < r0 + rn:
                n = min(SEG, r0 + rn - c)
                raw.append((c, n, base))
                c += n
        segs = []
        for (c, n, base) in sorted(raw, key=lambda x: -x[1]):
            q = min(QMAP, key=lambda qq: qload[qq])
            qload[q] += n
            segs.append((c, n, base, q))
        segs = [x for x in segs if x[3] != 0] + [x for x in segs if x[3] == 0]
        groups.append({"blocks": blocks, "goff": goff, "gch": gch,
                       "runs": runs, "segs": segs})
        goff = ch
    total_ch = goff

    idx_streams, mask_streams = [], []
    for c in range(N_CORES):
        sel = core == c
        e_row = src_row[sel]
        e_loc = local[sel]
        e_cls = cls[sel]
        e_slot = slot_of[c, e_loc]
        e_blk = e_slot // BLK

        # per-slot class counts and greedy run-fill quotas for this core
        cn = cnt[:, c, :]                      # [5, SHARD]
        K0s = K0[slot_of[c] // BLK]
        K1s = K1[slot_of[c] // BLK]
        t0_01 = np.minimum(cn[1], np.maximum(0, K0s - cn[0]))
        t0_012 = np.minimum(cn[2], np.maximum(0, K0s - cn[0] - t0_01))
        t1_01 = cn[1] - t0_01
        t1_12 = np.minimum(cn[3], np.maximum(0, K1s - t1_01))
        t1_012 = np.minimum(cn[2] - t0_012,
                            np.maximum(0, K1s - t1_01 - t1_12))
        rem12 = cn[3] - t1_12
        rem012 = cn[2] - t0_012 - t1_012
        assert (rem12 + rem012 + cn[4] <= K2[slot_of[c] // BLK] + 0).all()

        o = np.lexsort((e_cls, e_slot))
        r = _running_count(e_slot[o] * 8 + e_cls[o])  # rank in (slot, class)
        sl = e_loc[o]
        ecl = e_cls[o]
        run = np.zeros(len(o), np.int64)
        pos = np.zeros(len(o), np.int64)
        m = ecl == 0
        run[m], pos[m] = 0, r[m]
        m = ecl == 1
        in0 = r[m] < t0_01[sl[m]]
        run[m] = np.where(in0, 0, 1)
        pos[m] = np.where(in0, cn[0][sl[m]] + r[m], r[m] - t0_01[sl[m]])
        m = ecl == 2
        rr = r[m]
        in0 = rr < t0_012[sl[m]]
        in1 = (~in0) & (rr - t0_012[sl[m]] < t1_012[sl[m]])
        run[m] = np.where(in0, 0, np.where(in1, 1, 2))
        pos[m] = np.where(
            in0, cn[0][sl[m]] + t0_01[sl[m]] + rr,
            np.where(in1,
                     t1_01[sl[m]] + t1_12[sl[m]] + (rr - t0_012[sl[m]]),
                     rem12[sl[m]] + (rr - t0_012[sl[m]] - t1_012[sl[m]])))
        m = ecl == 3
        in1 = r[m] < t1_12[sl[m]]
        run[m] = np.where(in1, 1, 2)
        pos[m] = np.where(in1, t1_01[sl[m]] + r[m], r[m] - t1_12[sl[m]])
        m = ecl == 4
        run[m] = 2
        pos[m] = rem12[sl[m]] + rem012[sl[m]] + r[m]

        blk_o = e_blk[o]
        chunk = run_start[blk_o, run] + pos
        slots = chunk * BLK + (e_slot[o] % BLK)
        rows_o = e_row[o]
        basev = np.array(BASES, np.int64)[run]
        assert (rows_o - basev >= 0).all() and (rows_o - basev < SPLIT).all()

        idx = np.zeros(total_ch * BLK, np.int16)
        mask = np.zeros(total_ch * BLK, np.float32)
        idx[slots] = (rows_o - basev).astype(np.int16)
        mask[slots] = 1.0

        idx_w = np.tile(idx.reshape(total_ch * 8, 16).T, (8, 1)).copy()
        mask_w = mask.reshape(total_ch, BLK).T.astype(bf16).copy()
        idx_streams.append(idx_w)
        mask_streams.append(mask_w)

    plan.tot = T
    plan.groups = groups
    plan.total_ch = total_ch
    plan.idx_streams = idx_streams
    plan.mask_streams = mask_streams
    plan.perm = perm
    return plan


def _prep(edge_index):
    """One common slot permutation; x is host-permuted into slot order, so
    both layers share one table row space and ONE plan (identical streams)."""
    src = np.asarray(edge_index[0], dtype=np.int64)
    dst = np.asarray(edge_index[1], dtype=np.int64)
    # self-loops are handled analytically on-chip, not in the edge streams

    row_pm = _row_of_piece_major()
    s_core = src // SHARD
    s_local = src - s_core * SHARD
    core = dst // SHARD
    local = dst - core * SHARD
    deg = np.zeros((N_CORES, SHARD), np.int64)
    np.add.at(deg, (core, local), 1)

    order = np.argsort(-deg, axis=1, kind="stable")
    for _ in range(2):
        slot_of = np.zeros((N_CORES, SHARD), np.int64)
        for c in range(N_CORES):
            slot_of[c, order[c]] = np.arange(SHARD)
        srow = row_pm[s_core, slot_of[s_core, s_local]]
        klo_f = np.zeros((N_CORES, SHARD), np.int64)
        np.add.at(klo_f, (core, local), (srow < BASE1).astype(np.int64))
        order = np.lexsort((-klo_f, -deg), axis=-1)

    perm = np.full((N_CORES, NB * BLK), -1, np.int64)
    slot_of = np.zeros((N_CORES, SHARD), np.int64)
    for c in range(N_CORES):
        perm[c, :SHARD] = order[c]
        slot_of[c, order[c]] = np.arange(SHARD)
    src_row = row_pm[s_core, slot_of[s_core, s_local]]
    plan = _plan_layer(src_row, dst, perm, slot_of)
    return plan, plan


def _piece_bounds():
    """NPIECE block ranges aligned to group (SBG) boundaries."""
    ngroups = _ceil(NB, SBG)
    gb = [round(i * ngroups / NPIECE) for i in range(NPIECE + 1)]
    return [(gb[i], gb[i + 1], min(gb[i] * SBG, NB), min(gb[i + 1] * SBG, NB))
            for i in range(NPIECE)]


def _piece_rows():
    """Local-row ranges per piece and piece-major t_full bases (in rows)."""
    pr = [min(be * BLK, SHARD) for (_, _, _, be) in _piece_bounds()]
    pr = [0] + pr
    base = [N_CORES * r for r in pr]
    return pr, base


def _row_of_piece_major():
    """row_of[c, local] = piece-major t_full row of core c's local row."""
    pr, base = _piece_rows()
    row_of = np.zeros((N_CORES, SHARD), np.int64)
    for p in range(NPIECE):
        r0, r1 = pr[p], pr[p + 1]
        ln = r1 - r0
        for c in range(N_CORES):
            row_of[c, r0:r1] = base[p] + c * ln + np.arange(ln)
    return row_of


def _dma_blocks_out(nc, shard_dram, tab_sb, bs, be):
    """DMA tab_sb[:, bs:be, :] to shard_dram rows [bs*BLK, min(be*BLK, SHARD))."""
    r0 = bs * BLK
    r1 = min(be * BLK, SHARD)
    full = (r1 - r0) // BLK  # full blocks
    if full > 0:
        nc.scalar.dma_start(
            out=shard_dram[r0:r0 + full * BLK, :].rearrange(
                "(b p) c -> p b c", p=BLK, b=full),
            in_=tab_sb[:, bs:bs + full, :])
    rem = (r1 - r0) - full * BLK
    if rem > 0:
        nc.scalar.dma_start(out=shard_dram[r0 + full * BLK:r1, :],
                          in_=tab_sb[0:rem, bs + full, :])


AG_MODE = int(os.environ.get("K_AG_MODE", "3"))
GQMAP = [int(c) for c in os.environ.get("K_QMAP", "1230")]


def _ag_piece(nc, shard_dram, full_dram, bs, be, ncols, rg):
    """AllGather rows [bs*BLK, min(be*BLK, SHARD)) x cols [0:ncols)."""
    if AG_MODE == 0:
        return
    r0 = bs * BLK
    r1 = min(be * BLK, SHARD)
    if AG_MODE == 3:
        pr, base = _piece_rows()
        p = pr.index(r0)
        assert pr[p + 1] == r1
        nc.gpsimd.collective_compute(
            "AllGather", mybir.AluOpType.bypass, replica_groups=rg,
            ins=[shard_dram[r0:r1, :].opt()],
            outs=[full_dram[base[p]:base[p + 1], :].opt()])
        return
    full_v = full_dram.ap().rearrange("(r n) c -> r n c", r=N_CORES)
    if AG_MODE == 2:
        ncols = TROW
    nc.gpsimd.collective_compute(
        "AllGather", mybir.AluOpType.bypass, replica_groups=rg,
        ins=[shard_dram[r0:r1, 0:ncols]],
        outs=[full_v[:, r0:r1, 0:ncols]])


def _ag_whole(nc, shard_dram, full_dram, rg):
    if AG_MODE != 0:
        return
    nc.gpsimd.collective_compute(
        "AllGather", mybir.AluOpType.bypass, replica_groups=rg,
        ins=[shard_dram.ap().opt()], outs=[full_dram.ap().opt()])


def _build(plan1, plan2):
    nc = bacc.Bacc("TRN2", target_bir_lowering=False, debug=False,
                   num_devices=N_CORES, num_swdge_queues=4)

    NPADROWS = NB * BLK  # 6272
    xT_ext = nc.declare_dram_parameter("xT", [F_IN, NPADROWS], bfl, isOutput=False)
    w1_ext = nc.declare_dram_parameter("w1r", [128, 4 * D1], bfl, isOutput=False)
    w2_ext = nc.declare_dram_parameter("w2", [D1, C2], bfl, isOutput=False)
    a1s_ext = nc.declare_dram_parameter("a1srep", [128, D1], f32, isOutput=False)
    a1d_ext = nc.declare_dram_parameter("a1drep", [128, D1], f32, isOutput=False)
    a2s_ext = nc.declare_dram_parameter("a2srep", [128, C2], f32, isOutput=False)
    a2d_ext = nc.declare_dram_parameter("a2drep", [128, C2], f32, isOutput=False)
    b1_ext = nc.declare_dram_parameter("b1rep", [128, D1], f32, isOutput=False)
    b2_ext = nc.declare_dram_parameter("b2rep", [128, C2], f32, isOutput=False)
    idx1_ext = nc.declare_dram_parameter("idx1", [128, plan1.total_ch * 8], i16,
                                         isOutput=False)
    msk1_ext = nc.declare_dram_parameter("msk1", [128, plan1.total_ch], bfl,
                                         isOutput=False)
    out_ext = nc.declare_dram_parameter("out", [NB * BLK, C2], f32, isOutput=True)

    t1_shard = nc.dram_tensor("t1_shard", [SHARD, TROW], bfl)
    t1_full = nc.dram_tensor("t1_full", [N_NODES, TROW], bfl, addr_space="Shared")
    t2_shard = nc.dram_tensor("t2_shard", [SHARD, TROW], bfl)
    t2_full = nc.dram_tensor("t2_full", [N_NODES, TROW], bfl, addr_space="Shared")

    rg = [list(range(N_CORES))]
    pieces = _piece_bounds()

    with tile.TileContext(nc) as tc:
        with tc.tile_pool(name="const", bufs=1) as cpool:
            ident = cpool.tile([128, 128], bfl)
            make_identity(nc, ident[:, :])
            a1s_t = cpool.tile([128, D1], f32)
            nc.sync.dma_start(out=a1s_t[:, :], in_=a1s_ext[:, :])
            a1d_t = cpool.tile([128, D1], f32)
            nc.sync.dma_start(out=a1d_t[:, :], in_=a1d_ext[:, :])
            a2s_t = cpool.tile([128, C2], f32)
            nc.sync.dma_start(out=a2s_t[:, :], in_=a2s_ext[:, :])
            a2d_t = cpool.tile([128, C2], f32)
            nc.sync.dma_start(out=a2d_t[:, :], in_=a2d_ext[:, :])
            b1_t = cpool.tile([128, D1], f32)
            nc.sync.dma_start(out=b1_t[:, :], in_=b1_ext[:, :])
            b2_t = cpool.tile([128, C2], f32)
            nc.sync.dma_start(out=b2_t[:, :], in_=b2_ext[:, :])
            w2_t = cpool.tile([D1, C2], bfl)
            nc.sync.dma_start(out=w2_t[:, :], in_=w2_ext[:, :])
            tab1_sb = cpool.tile([128, NB, TROW], bfl)
            tab2_sb = cpool.tile([128, NB, TROW], bfl)
            nc.vector.memset(tab1_sb[:, :, :], 0.0)
            nc.vector.memset(tab2_sb[:, :, :], 0.0)
            idx_all = cpool.tile([128, plan1.total_ch * 8], i16)
            nc.sync.dma_start(out=idx_all[:, :], in_=idx1_ext[:, :])
            msk_all = cpool.tile([128, plan1.total_ch], bfl)
            nc.sync.dma_start(out=msk_all[:, :], in_=msk1_ext[:, :])


            # ---- Phase A: h1 = x @ W1 + attention scalars, piecewise AG1
            with tc.tile_pool(name="phA", bufs=2) as apool, \
                 tc.tile_pool(name="phA_ps", bufs=4, space="PSUM") as apsum:
                w1_t = apool.tile([128, 4, D1], bfl, tag="w1")
                nc.sync.dma_start(out=w1_t[:, :, :], in_=w1_ext[:, :])
                xk = []
                for k in range(4):
                    xt = apool.tile([128, NPADROWS], bfl, tag=f"xk{k}")
                    nc.sync.dma_start(out=xt[:, :],
                                      in_=xT_ext[k * 128:(k + 1) * 128, :])
                    xk.append(xt)
                for (g0, g1, bs, be) in pieces:
                    for b in range(bs, be):
                        hps = apsum.tile([128, D1], f32, tag="hps")
                        for k in range(4):
                            nc.tensor.matmul(
                                hps[:, :],
                                lhsT=xk[k][:, b * BLK:(b + 1) * BLK],
                                rhs=w1_t[:, k, :], start=(k == 0), stop=(k == 3))
                        nc.scalar.activation(
                            out=tab1_sb[:, b, 0:D1], in_=hps[:, :],
                            func=mybir.ActivationFunctionType.Copy)
                        for a_t, sl in ((a1s_t, T1_AS), (a1d_t, T1_AD)):
                            tmp = apool.tile([128, D1], f32, tag="atmp")
                            nc.vector.tensor_tensor(
                                out=tmp[:, :], in0=hps[:, :], in1=a_t[:, :],
                                op=mybir.AluOpType.mult)
                            nc.vector.tensor_reduce(
                                out=tab1_sb[:, b, sl[0]:sl[1]].bitcast(f32),
                                in_=tmp[:, :].rearrange(
                                    "p (h c) -> p h c", h=H1, c=HID),
                                axis=mybir.AxisListType.X,
                                op=mybir.AluOpType.add)
                    _dma_blocks_out(nc, t1_shard, tab1_sb, bs, be)
                    _ag_piece(nc, t1_shard, t1_full, bs, be, T1_AGC, rg)

            _ag_whole(nc, t1_shard, t1_full, rg)

            _edge_phase(nc, tc, layer=1, table_full=t1_full,
                        idx_all=idx_all, msk_all=msk_all, ident=ident,
                        plan=plan1, pieces=pieces, bias=b1_t, w2_t=w2_t,
                        a2s_t=a2s_t, a2d_t=a2d_t, tab_sb=tab1_sb,
                        tab_out=tab2_sb, t_shard=t2_shard, t_full=t2_full,
                        rg=rg, out_ext=None, b2_t=None)

            _ag_whole(nc, t2_shard, t2_full, rg)

            _edge_phase(nc, tc, layer=2, table_full=t2_full,
                        idx_all=idx_all, msk_all=msk_all, ident=ident,
                        plan=plan2, pieces=pieces, bias=None, w2_t=None,
                        a2s_t=None, a2d_t=None, tab_sb=tab2_sb,
                        tab_out=None, t_shard=None, t_full=None,
                        rg=rg, out_ext=out_ext, b2_t=b2_t)

    nc.compile()
    return nc


def _edge_phase(nc, tc, layer, table_full, idx_all, msk_all, ident,
                plan, pieces, bias, w2_t, a2s_t, a2d_t, tab_sb,
                tab_out, t_shard, t_full, rg, out_ext, b2_t):
    if layer == 1:
        NH, CH, CC = H1, HID, D1
        asrc_sl, adst_sl = T1_AS, T1_AD
    else:
        NH, CH, CC = 1, C2, C2
        asrc_sl, adst_sl = T2_AS, T2_AD
    NCOL = CC + NH
    gmax = max(g["gch"] for g in plan.groups)

    with tc.tile_pool(name=f"self{layer}", bufs=1) as spool, \
         tc.tile_pool(name=f"e{layer}", bufs=2) as pool, \
         tc.tile_pool(name=f"e{layer}_ps", bufs=2, space="PSUM") as psum, \
         tc.tile_pool(name=f"e{layer}_ps2", bufs=2, space="PSUM") as psum2:
        # analytic self-loop term: w = exp(leakyrelu(a_src[d]+a_dst[d])),
        # num += w*h[d], den += w  (self-loops excluded from edge streams)
        es = spool.tile([128, NB, NH], f32)
        nc.vector.tensor_tensor(
            out=es[:, :, :],
            in0=tab_sb[:, :, asrc_sl[0]:asrc_sl[1]].bitcast(f32),
            in1=tab_sb[:, :, adst_sl[0]:adst_sl[1]].bitcast(f32),
            op=mybir.AluOpType.add)
        lrs = spool.tile([128, NB, NH], f32)
        nc.vector.scalar_tensor_tensor(
            out=lrs[:, :, :], in0=es[:, :, :], scalar=NEG_SLOPE,
            in1=es[:, :, :], op0=mybir.AluOpType.mult,
            op1=mybir.AluOpType.max)
        ws = spool.tile([128, NB, NH], f32)
        nc.scalar.activation(out=ws[:, :, :], in_=lrs[:, :, :],
                             func=mybir.ActivationFunctionType.Exp)
        sn = spool.tile([128, NB, CC], f32)
        nc.vector.tensor_tensor(
            out=sn[:, :, :].rearrange("p b (h c) -> p b h c", h=NH, c=CH),
            in0=tab_sb[:, :, 0:CC].rearrange("p b (h c) -> p b h c",
                                             h=NH, c=CH),
            in1=ws[:, :, :, None].to_broadcast([128, NB, NH, CH]),
            op=mybir.AluOpType.mult)
        piece_of_group = {}
        for pi, (g0, g1, bs, be) in enumerate(pieces):
            for g in range(g0, g1):
                piece_of_group[g] = pi if g == g1 - 1 else None

        for gi, grp in enumerate(plan.groups):
            goff, gch = grp["goff"], grp["gch"]
            mskg = msk_all[:, goff:goff + gch]
            g_t = pool.tile([128, gmax, TROW], bfl, tag="gath")
            for (c0, n, base, q) in grp["segs"]:
                in_ap = table_full[BASES[base]:BASES[base] + SPLIT, :]
                nc.gpsimd.dma_gather(
                    out_ap=g_t[:, c0:c0 + n, :], in_ap=in_ap,
                    idxs_ap=idx_all[:, (goff + c0) * 8:(goff + c0 + n) * 8],
                    num_idxs=n * BLK, num_idxs_reg=n * BLK,
                    elem_size=TROW, single_packet=False, queue_num=q)

            # group-wide attention pipeline
            e_t = pool.tile([128, gmax, NH], f32, tag="elog")
            for b, (l0, rn) in zip(grp["blocks"], grp["runs"]):
                nc.vector.tensor_tensor(
                    out=e_t[:, l0:l0 + rn, :],
                    in0=g_t[:, l0:l0 + rn,
                            asrc_sl[0]:asrc_sl[1]].bitcast(f32),
                    in1=tab_sb[:, b, adst_sl[0]:adst_sl[1]].bitcast(f32)
                        [:, None, :].to_broadcast([128, rn, NH]),
                    op=mybir.AluOpType.add)
            lr_t = pool.tile([128, gmax, NH], f32, tag="lrt")
            nc.vector.scalar_tensor_tensor(
                out=lr_t[:, 0:gch, :], in0=e_t[:, 0:gch, :], scalar=NEG_SLOPE,
                in1=e_t[:, 0:gch, :], op0=mybir.AluOpType.mult,
                op1=mybir.AluOpType.max)
            exf = pool.tile([128, gmax, NH], f32, tag="exf")
            nc.scalar.activation(out=exf[:, 0:gch, :], in_=lr_t[:, 0:gch, :],
                                 func=mybir.ActivationFunctionType.Exp)
            r_t = pool.tile([128, gmax, NCOL], bfl, tag="rmat")
            nc.vector.tensor_tensor(
                out=r_t[:, 0:gch, CC:NCOL], in0=exf[:, 0:gch, :],
                in1=mskg[:, :, None].to_broadcast([128, gch, NH]),
                op=mybir.AluOpType.mult)
            nc.vector.tensor_tensor(
                out=r_t[:, 0:gch, 0:CC].rearrange("p g (h c) -> p g h c",
                                                  h=NH, c=CH),
                in0=g_t[:, 0:gch, 0:CC].rearrange("p g (h c) -> p g h c",
                                                  h=NH, c=CH),
                in1=r_t[:, 0:gch, CC:NCOL, None].to_broadcast(
                    [128, gch, NH, CH]),
                op=mybir.AluOpType.mult)

            for b, (l0, rn) in zip(grp["blocks"], grp["runs"]):
                ps = psum.tile([128, NCOL], f32, tag="agg")
                for j in range(rn):
                    nc.tensor.matmul(ps[:, :], lhsT=ident[:, :],
                                     rhs=r_t[:, l0 + j, :], start=(j == 0),
                                     stop=(j == rn - 1))
                den = pool.tile([128, NH], f32, tag="den")
                nc.vector.scalar_tensor_tensor(
                    out=den[:, :], in0=ps[:, CC:NCOL], scalar=1e-16,
                    in1=ws[:, b, :], op0=mybir.AluOpType.add,
                    op1=mybir.AluOpType.add)
                recip = pool.tile([128, NH], f32, tag="recip")
                nc.vector.reciprocal(out=recip[:, :], in_=den[:, :])
                onum = pool.tile([128, CC], f32, tag="onum")
                nc.vector.tensor_tensor(out=onum[:, :], in0=ps[:, 0:CC],
                                        in1=sn[:, b, :],
                                        op=mybir.AluOpType.add)
                o_t = pool.tile([128, CC], f32, tag="outb")
                nc.vector.tensor_tensor(
                    out=o_t[:, :].rearrange("p (h c) -> p h c", h=NH, c=CH),
                    in0=onum[:, :].rearrange("p (h c) -> p h c", h=NH, c=CH),
                    in1=recip[:, :, None].to_broadcast([128, NH, CH]),
                    op=mybir.AluOpType.mult)

                if layer == 1:
                    obt = pool.tile([128, CC], f32, tag="outbt")
                    nc.vector.tensor_tensor(out=obt[:, :], in0=o_t[:, :],
                                            in1=bias[:, :],
                                            op=mybir.AluOpType.add)
                    ob = pool.tile([128, CC], bfl, tag="outbf")
                    nc.vector.tensor_scalar(out=ob[:, :], in0=obt[:, :],
                                            scalar1=0.0, scalar2=None,
                                            op0=mybir.AluOpType.max)
                    tps = psum2.tile([D1, 128], bfl, tag="tp")
                    nc.tensor.transpose(tps[:, :], ob[:, :], ident[:, :])
                    h1T = pool.tile([D1, 128], bfl, tag="h1T")
                    nc.vector.tensor_copy(out=h1T[:, :], in_=tps[:, :])
                    h2ps = psum2.tile([128, C2], f32, tag="h2")
                    nc.tensor.matmul(h2ps[:, :], lhsT=h1T[:, :], rhs=w2_t[:, :],
                                     start=True, stop=True)
                    nc.scalar.activation(out=tab_out[:, b, 0:C2],
                                         in_=h2ps[:, :],
                                         func=mybir.ActivationFunctionType.Copy)
                    for a_t, sl in ((a2s_t, T2_AS), (a2d_t, T2_AD)):
                        t2a = pool.tile([128, C2], f32, tag="t2a")
                        nc.vector.tensor_tensor(out=t2a[:, :], in0=h2ps[:, :],
                                                in1=a_t[:, :],
                                                op=mybir.AluOpType.mult)
                        nc.vector.tensor_reduce(
                            out=tab_out[:, b, sl[0]:sl[1]].bitcast(f32),
                            in_=t2a[:, :], axis=mybir.AxisListType.X,
                            op=mybir.AluOpType.add)
                else:
                    lg = pool.tile([128, C2], f32, tag="logits")
                    nc.vector.tensor_tensor(out=lg[:, :], in0=o_t[:, :],
                                            in1=b2_t[:, :],
                                            op=mybir.AluOpType.add)
                    negm = pool.tile([128, 1], f32, tag="negm")
                    nc.vector.tensor_reduce(out=negm[:, :], in_=lg[:, :],
                                            axis=mybir.AxisListType.X,
                                            op=mybir.AluOpType.max, negate=True)
                    ex = pool.tile([128, C2], f32, tag="sfex")
                    ssum = pool.tile([128, 1], f32, tag="ssum")
                    nc.scalar.activation(out=ex[:, :], in_=lg[:, :],
                                         func=mybir.ActivationFunctionType.Exp,
                                         bias=negm[:, :], accum_out=ssum[:, :])
                    lse = pool.tile([128, 1], f32, tag="lse")
                    nc.scalar.activation(out=lse[:, :], in_=ssum[:, :],
                                         func=mybir.ActivationFunctionType.Ln)
                    res = pool.tile([128, C2], f32, tag="res")
                    nc.vector.scalar_tensor_tensor(
                        out=res[:, :], in0=lg[:, :], scalar=negm[:, :],
                        in1=lse[:, :].to_broadcast([128, C2]),
                        op0=mybir.AluOpType.add, op1=mybir.AluOpType.subtract)
                    nc.scalar.dma_start(out=out_ext[b * BLK:(b + 1) * BLK, :],
                                      in_=res[:, :])

            if layer == 1:
                pi = piece_of_group.get(gi)
                if pi is not None:
                    g0, g1, bs, be = pieces[pi]
                    _dma_blocks_out(nc, t_shard, tab_out, bs, be)
                    _ag_piece(nc, t_shard, t_full, bs, be, T2_AGC, rg)


def _host_inputs(x, W1, att_src1, att_dst1, b1, W2, att_src2, att_dst2, b2,
                 plan1, plan2):
    NPADROWS = NB * BLK
    w1r = np.ascontiguousarray(
        np.asarray(W1, np.float32).reshape(4, 128, D1).transpose(1, 0, 2)
    ).reshape(128, 4 * D1).astype(bf16)
    rep = lambda v, n: np.tile(np.asarray(v, np.float32).reshape(1, n),
                               (128, 1)).astype(np.float32)
    x32 = np.asarray(x, np.float32)

    in_maps = []
    for c in range(N_CORES):
        xs = x32[c * SHARD:(c + 1) * SHARD][plan1.perm[c, :SHARD]]
        xT = np.zeros((F_IN, NPADROWS), bf16)
        xT[:, :SHARD] = xs.T.astype(bf16)
        in_maps.append({
            "xT": xT,
            "w1r": w1r,
            "w2": np.asarray(W2, np.float32).astype(bf16),
            "a1srep": rep(att_src1, D1),
            "a1drep": rep(att_dst1, D1),
            "a2srep": rep(att_src2, C2),
            "a2drep": rep(att_dst2, C2),
            "b1rep": rep(b1, D1),
            "b2rep": rep(b2, C2),
            "idx1": plan1.idx_streams[c],
            "msk1": plan1.mask_streams[c],
        })
    return in_maps


def kernel_run(inputs, trace=False):
    edge_index = inputs["edge_index"]
    plan1, plan2 = _prep(edge_index)

    key = tuple(plan1.tot)
    if key not in _CACHE:
        _CACHE[key] = _build(plan1, plan2)
    nc = _CACHE[key]

    in_maps = _host_inputs(
        inputs["x"], inputs["W1"], inputs["att_src1"], inputs["att_dst1"],
        inputs["b1"], inputs["W2"], inputs["att_src2"], inputs["att_dst2"],
        inputs["b2"], plan1, plan2)

    if trace:
        _install_ntff_hook()
    res = run_bass_kernel_spmd(nc, in_maps, core_ids=list(range(N_CORES)),
                               trace=trace)
    out = np.zeros((N_NODES, C2), np.float32)
    for c in range(N_CORES):
        o = res.results[c]["out"]
        mem = plan2.perm[c]
        valid = mem >= 0
        out[c * SHARD + mem[valid]] = o[valid]
    return out, res.exec_time_ns


def kernel(**inputs):
    out, _ = kernel_run(inputs)
    return out


# revision 6
# speedup vs baseline: 1.8842x; 1.0211x over previous
"""GAT 2-layer GNN kernel for 8 Trainium2 NeuronCores — v2.

Structure (vs the v1 baseline):
  - Nodes partitioned into 8 shards of 6250; per-core node features +
    attention scalars packed into 256-byte table rows; AllGather replicates
    the table (column-sliced: only the bytes the edge phase reads travel,
    and the AllGather is issued in 4 block-range pieces overlapped with
    table production).
  - Edges in ELL layout keyed by dst: dst slot = SBUF partition, per-dst
    edge list split into a lo run (table row < 32768) and a hi run, laid
    out per-block-contiguously [b0.lo|b0.hi|b1.lo|b1.hi|...] so each
    block's chunks are one contiguous range.
  - h[src]/a_src[src] fetched per-edge with dma_gather. Descriptor
    generation (~7.8ns/idx on the Q7) is the kernel bottleneck, so each
    group's gather is split into ~16-chunk segments spread over all 4
    SWDGE queues; queues 1-3 generate asynchronously, queue 0 inline on
    the Pool engine, giving ~4x parallel descriptor generation.
  - a_dst[dst] is never gathered: it stays on-chip in the per-core table
    SBUF copy (tab1_sb/tab2_sb) written during production.
  - Vector work is batched group-wide ([128, gch, H] ops) instead of
    per-run; the segment softmax-aggregate is identity-lhsT PSUM matmuls
    per chunk as before.
"""

import os
import sys

sys.path.insert(0, "/opt/trn_rl_repo")

import numpy as np
import ml_dtypes

import concourse.bacc as bacc
import concourse.mybir as mybir
from concourse import tile
from concourse.bass_utils import run_bass_kernel_spmd
from concourse.masks import make_identity

bf16 = ml_dtypes.bfloat16

N_NODES = 50000
F_IN = 512
H1 = 8
HID = 8
D1 = H1 * HID  # 64
C2 = 40
N_CORES = 8
SHARD = N_NODES // N_CORES  # 6250
BLK = 128
NB = (SHARD + BLK - 1) // BLK  # 49
SPLIT = 32768
HIBASE = N_NODES - SPLIT  # 17232
BASE1 = 8616  # middle gather base; BASES rows: [0,32768) [8616,41384) [17232,50000)
BASES = (0, BASE1, HIBASE)
SBG = 2  # blocks per super-group
SEG = int(os.environ.get("K_SEG", "24"))  # max chunks per gather segment
NEG_SLOPE = 0.2
TROW = 128  # table row: 128 bf16 = 256 bytes
NPIECE = 4

f32 = mybir.dt.float32
bfl = mybir.dt.bfloat16
i16 = mybir.dt.int16

# table col layouts (bf16 col units)
T1_AS = (64, 80)   # a_src1: 8 x f32
T1_AD = (80, 96)   # a_dst1: 8 x f32 (local only, not gathered/AG'd)
T1_AGC = 80        # AllGather cols 0:80 (160B)
T2_AS = (40, 42)   # a_src2: 1 x f32
T2_AD = (42, 44)   # a_dst2: local only
T2_AGC = 42        # AllGather cols 0:42 (84B)

_CACHE = {}


def _install_ntff_hook():
    """Provide antenv.axon_hooks if the image lacks it (NTFF profiling)."""
    try:
        from antenv.axon_hooks import get_axon_ntff_profile_hook  # noqa: F401
        return
    except ImportError:
        pass
    import contextlib
    import ctypes
    import types

    so_path = "/opt/axon/libaxon_pjrt.so"
    try:
        lib = ctypes.CDLL(so_path)
    except OSError:
        return
    if not hasattr(lib, "axon_start_nrt_profile"):
        return
    lib.axon_start_nrt_profile.argtypes = [ctypes.POINTER(ctypes.c_int64),
                                           ctypes.c_size_t]
    lib.axon_start_nrt_profile.restype = ctypes.c_int64
    lib.axon_stop_nrt_profile.argtypes = [ctypes.c_char_p]
    lib.axon_stop_nrt_profile.restype = ctypes.c_int64

    @contextlib.contextmanager
    def _hook(output_dir, device_ids):
        import jax
        jax.devices()
        if device_ids:
            ids = (ctypes.c_int64 * len(device_ids))(*device_ids)
            rc = lib.axon_start_nrt_profile(ids, len(device_ids))
        else:
            rc = lib.axon_start_nrt_profile(None, 0)
        if rc != 0:
            raise RuntimeError(f"axon_start_nrt_profile rc={rc}")
        try:
            yield
        finally:
            n = lib.axon_stop_nrt_profile(str(output_dir).encode())
            print(f"ntff profile: {n} file(s) written to {output_dir}")

    import antenv
    mod = types.ModuleType("antenv.axon_hooks")
    mod.get_axon_ntff_profile_hook = lambda: _hook
    mod.set_axon_ntff_profile_hook = lambda h: None
    sys.modules["antenv.axon_hooks"] = mod
    antenv.axon_hooks = mod


def _ceil(a, b):
    return (a + b - 1) // b


def _running_count(k):
    """pos[i] = number of j<i with k[j]==k[i]; k is sorted."""
    n = len(k)
    if n == 0:
        return np.zeros(0, np.int64)
    starts = np.r_[0, np.flatnonzero(np.diff(k)) + 1]
    run_id = np.zeros(n, np.int64)
    run_id[starts[1:]] = 1
    run_id = np.cumsum(run_id)
    return np.arange(n) - starts[run_id]


QMAP = [int(c) for c in os.environ.get("K_QMAP", "1230")]


class LayerPlan:
    pass


def _plan_layer(src_row, dst_node, perm, slot_of):
    """ELL plan with a 3-base flexible split: gather bases at rows 0, 8616,
    17232 (each covering 32768 rows). Every row is reachable from >=2 bases,
    so per-block run quotas K0/K1/K2 pack each block near its max-degree
    bound. perm/slot_of: the common slot permutation."""
    plan = LayerPlan()
    core = dst_node // SHARD
    local = dst_node - core * SHARD
    # classes: 0:[0,B1) run0 | 1:[B1,HIBASE) runs01 | 2:[HIBASE,SPLIT) any
    #          3:[SPLIT,B1+SPLIT) runs12 | 4:[B1+SPLIT,N) run2
    cls = np.digitize(src_row, [BASE1, HIBASE, SPLIT, BASE1 + SPLIT])

    cnt = np.zeros((5, N_CORES, SHARD), np.int64)
    for k in range(5):
        np.add.at(cnt[k], (core, local), (cls == k).astype(np.int64))
    n0, n01, n012, n12, n2 = cnt
    deg = cnt.sum(axis=0)

    order = perm[:, :SHARD]
    pad = np.zeros((N_CORES, NB * BLK), np.int64)
    def blockmax(x):
        p = pad.copy()
        p[:, :SHARD] = np.take_along_axis(x, order, 1)
        return p.reshape(N_CORES, NB, BLK).max(axis=(0, 2))
    M0 = np.maximum(blockmax(n0), 1)
    M01 = blockmax(n0 + n01)
    M12b = blockmax(n12 + n2)
    M2 = np.maximum(blockmax(n2), 1)
    D = blockmax(deg)
    T = np.maximum.reduce([D, M01 + M2, M0 + M12b, M0 + M2 + 1,
                           np.full(NB, 3, np.int64)])
    K0 = M0
    K2 = M2
    K1 = np.maximum.reduce([M01 - K0, M12b - K2, np.ones(NB, np.int64)])
    K1 = K1 + (T - (K0 + K1 + K2))
    Ks = np.stack([K0, K1, K2])          # [3, NB]
    assert (Ks > 0).all() and (Ks.sum(axis=0) == T).all()

    run_start = np.zeros((NB, 3), np.int64)   # global chunk offset per run
    groups = []
    goff = 0
    qload = {q: 0 for q in QMAP}
    for g in range(_ceil(NB, SBG)):
        blocks = list(range(g * SBG, min((g + 1) * SBG, NB)))
        ch = goff
        runs = []
        segruns = []
        for b in blocks:
            l0 = ch - goff
            for r in range(3):
                run_start[b, r] = ch
                segruns.append((ch - goff, int(Ks[r, b]), r))
                ch += int(Ks[r, b])
            runs.append((l0, int(T[b])))
        gch = ch - goff
        raw = []
        for (r0, rn, base) in segruns:
            c = r0
            while c < r0 + rn:
                n = min(SEG, r0 + rn - c)
                raw.append((c, n, base))
                c += n
        segs = []
        for (c, n, base) in sorted(raw, key=lambda x: -x[1]):
            q = min(QMAP, key=lambda qq: qload[qq])
            qload[q] += n
            segs.append((c, n, base, q))
        segs = [x for x in segs if x[3] != 0] + [x for x in segs if x[3] == 0]
        groups.append({"blocks": blocks, "goff": goff, "gch": gch,
                       "runs": runs, "segs": segs})
        goff = ch
    total_ch = goff

    idx_streams, mask_streams = [], []
    for c in range(N_CORES):
        sel = core == c
        e_row = src_row[sel]
        e_loc = local[sel]
        e_cls = cls[sel]
        e_slot = slot_of[c, e_loc]
        e_blk = e_slot // BLK

        # per-slot class counts and greedy run-fill quotas for this core
        cn = cnt[:, c, :]                      # [5, SHARD]
        K0s = K0[slot_of[c] // BLK]
        K1s = K1[slot_of[c] // BLK]
        t0_01 = np.minimum(cn[1], np.maximum(0, K0s - cn[0]))
        t0_012 = np.minimum(cn[2], np.maximum(0, K0s - cn[0] - t0_01))
        t1_01 = cn[1] - t0_01
        t1_12 = np.minimum(cn[3], np.maximum(0, K1s - t1_01))
        t1_012 = np.minimum(cn[2] - t0_012,
                            np.maximum(0, K1s - t1_01 - t1_12))
        rem12 = cn[3] - t1_12
        rem012 = cn[2] - t0_012 - t1_012
        assert (rem12 + rem012 + cn[4] <= K2[slot_of[c] // BLK] + 0).all()

        o = np.lexsort((e_cls, e_slot))
        r = _running_count(e_slot[o] * 8 + e_cls[o])  # rank in (slot, class)
        sl = e_loc[o]
        ecl = e_cls[o]
        run = np.zeros(len(o), np.int64)
        pos = np.zeros(len(o), np.int64)
        m = ecl == 0
        run[m], pos[m] = 0, r[m]
        m = ecl == 1
        in0 = r[m] < t0_01[sl[m]]
        run[m] = np.where(in0, 0, 1)
        pos[m] = np.where(in0, cn[0][sl[m]] + r[m], r[m] - t0_01[sl[m]])
        m = ecl == 2
        rr = r[m]
        in0 = rr < t0_012[sl[m]]
        in1 = (~in0) & (rr - t0_012[sl[m]] < t1_012[sl[m]])
        run[m] = np.where(in0, 0, np.where(in1, 1, 2))
        pos[m] = np.where(
            in0, cn[0][sl[m]] + t0_01[sl[m]] + rr,
            np.where(in1,
                     t1_01[sl[m]] + t1_12[sl[m]] + (rr - t0_012[sl[m]]),
                     rem12[sl[m]] + (rr - t0_012[sl[m]] - t1_012[sl[m]])))
        m = ecl == 3
        in1 = r[m] < t1_12[sl[m]]
        run[m] = np.where(in1, 1, 2)
        pos[m] = np.where(in1, t1_01[sl[m]] + r[m], r[m] - t1_12[sl[m]])
        m = ecl == 4
        run[m] = 2
        pos[m] = rem12[sl[m]] + rem012[sl[m]] + r[m]

        blk_o = e_blk[o]
        chunk = run_start[blk_o, run] + pos
        slots = chunk * BLK + (e_slot[o] % BLK)
        rows_o = e_row[o]
        basev = np.array(BASES, np.int64)[run]
        assert (rows_o - basev >= 0).all() and (rows_o - basev < SPLIT).all()

        idx = np.zeros(total_ch * BLK, np.int16)
        mask = np.zeros(total_ch * BLK, np.float32)
        idx[slots] = (rows_o - basev).astype(np.int16)
        mask[slots] = 1.0

        idx_w = np.tile(idx.reshape(total_ch * 8, 16).T, (8, 1)).copy()
        mask_w = mask.reshape(total_ch, BLK).T.astype(bf16).copy()
        idx_streams.append(idx_w)
        mask_streams.append(mask_w)

    plan.tot = T
    plan.groups = groups
    plan.total_ch = total_ch
    plan.idx_streams = idx_streams
    plan.mask_streams = mask_streams
    plan.perm = perm
    return plan


def _prep(edge_index):
    """One common slot permutation; x is host-permuted into slot order, so
    both layers share one table row space and ONE plan (identical streams)."""
    src = np.asarray(edge_index[0], dtype=np.int64)
    dst = np.asarray(edge_index[1], dtype=np.int64)
    # self-loops are handled analytically on-chip, not in the edge streams

    row_pm = _row_of_piece_major()
    s_core = src // SHARD
    s_local = src - s_core * SHARD
    core = dst // SHARD
    local = dst - core * SHARD
    deg = np.zeros((N_CORES, SHARD), np.int64)
    np.add.at(deg, (core, local), 1)

    order = np.argsort(-deg, axis=1, kind="stable")
    for _ in range(2):
        slot_of = np.zeros((N_CORES, SHARD), np.int64)
        for c in range(N_CORES):
            slot_of[c, order[c]] = np.arange(SHARD)
        srow = row_pm[s_core, slot_of[s_core, s_local]]
        klo_f = np.zeros((N_CORES, SHARD), np.int64)
        np.add.at(klo_f, (core, local), (srow < BASE1).astype(np.int64))
        order = np.lexsort((-klo_f, -deg), axis=-1)

    perm = np.full((N_CORES, NB * BLK), -1, np.int64)
    slot_of = np.zeros((N_CORES, SHARD), np.int64)
    for c in range(N_CORES):
        perm[c, :SHARD] = order[c]
        slot_of[c, order[c]] = np.arange(SHARD)
    src_row = row_pm[s_core, slot_of[s_core, s_local]]
    plan = _plan_layer(src_row, dst, perm, slot_of)
    return plan, plan


def _piece_bounds():
    """NPIECE block ranges aligned to group (SBG) boundaries."""
    ngroups = _ceil(NB, SBG)
    gb = [round(i * ngroups / NPIECE) for i in range(NPIECE + 1)]
    return [(gb[i], gb[i + 1], min(gb[i] * SBG, NB), min(gb[i + 1] * SBG, NB))
            for i in range(NPIECE)]


def _piece_rows():
    """Local-row ranges per piece and piece-major t_full bases (in rows)."""
    pr = [min(be * BLK, SHARD) for (_, _, _, be) in _piece_bounds()]
    pr = [0] + pr
    base = [N_CORES * r for r in pr]
    return pr, base


def _row_of_piece_major():
    """row_of[c, local] = piece-major t_full row of core c's local row."""
    pr, base = _piece_rows()
    row_of = np.zeros((N_CORES, SHARD), np.int64)
    for p in range(NPIECE):
        r0, r1 = pr[p], pr[p + 1]
        ln = r1 - r0
        for c in range(N_CORES):
            row_of[c, r0:r1] = base[p] + c * ln + np.arange(ln)
    return row_of


def _dma_blocks_out(nc, shard_dram, tab_sb, bs, be):
    """DMA tab_sb[:, bs:be, :] to shard_dram rows [bs*BLK, min(be*BLK, SHARD))."""
    r0 = bs * BLK
    r1 = min(be * BLK, SHARD)
    full = (r1 - r0) // BLK  # full blocks
    if full > 0:
        nc.scalar.dma_start(
            out=shard_dram[r0:r0 + full * BLK, :].rearrange(
                "(b p) c -> p b c", p=BLK, b=full),
            in_=tab_sb[:, bs:bs + full, :])
    rem = (r1 - r0) - full * BLK
    if rem > 0:
        nc.scalar.dma_start(out=shard_dram[r0 + full * BLK:r1, :],
                          in_=tab_sb[0:rem, bs + full, :])


AG_MODE = int(os.environ.get("K_AG_MODE", "3"))
GQMAP = [int(c) for c in os.environ.get("K_QMAP", "1230")]


def _ag_piece(nc, shard_dram, full_dram, bs, be, ncols, rg):
    """AllGather rows [bs*BLK, min(be*BLK, SHARD)) x cols [0:ncols)."""
    if AG_MODE == 0:
        return
    r0 = bs * BLK
    r1 = min(be * BLK, SHARD)
    if AG_MODE == 3:
        pr, base = _piece_rows()
        p = pr.index(r0)
        assert pr[p + 1] == r1
        nc.gpsimd.collective_compute(
            "AllGather", mybir.AluOpType.bypass, replica_groups=rg,
            ins=[shard_dram[r0:r1, :].opt()],
            outs=[full_dram[base[p]:base[p + 1], :].opt()])
        return
    full_v = full_dram.ap().rearrange("(r n) c -> r n c", r=N_CORES)
    if AG_MODE == 2:
        ncols = TROW
    nc.gpsimd.collective_compute(
        "AllGather", mybir.AluOpType.bypass, replica_groups=rg,
        ins=[shard_dram[r0:r1, 0:ncols]],
        outs=[full_v[:, r0:r1, 0:ncols]])


def _ag_whole(nc, shard_dram, full_dram, rg):
    if AG_MODE != 0:
        return
    nc.gpsimd.collective_compute(
        "AllGather", mybir.AluOpType.bypass, replica_groups=rg,
        ins=[shard_dram.ap().opt()], outs=[full_dram.ap().opt()])


def _build(plan1, plan2):
    nc = bacc.Bacc("TRN2", target_bir_lowering=False, debug=False,
                   num_devices=N_CORES, num_swdge_queues=4)

    NPADROWS = NB * BLK  # 6272
    xT_ext = nc.declare_dram_parameter("xT", [F_IN, NPADROWS], bfl, isOutput=False)
    w1_ext = nc.declare_dram_parameter("w1r", [128, 4 * D1], bfl, isOutput=False)
    w2_ext = nc.declare_dram_parameter("w2", [D1, C2], bfl, isOutput=False)
    a1s_ext = nc.declare_dram_parameter("a1srep", [128, D1], f32, isOutput=False)
    a1d_ext = nc.declare_dram_parameter("a1drep", [128, D1], f32, isOutput=False)
    a2s_ext = nc.declare_dram_parameter("a2srep", [128, C2], f32, isOutput=False)
    a2d_ext = nc.declare_dram_parameter("a2drep", [128, C2], f32, isOutput=False)
    b1_ext = nc.declare_dram_parameter("b1rep", [128, D1], f32, isOutput=False)
    b2_ext = nc.declare_dram_parameter("b2rep", [128, C2], f32, isOutput=False)
    idx1_ext = nc.declare_dram_parameter("idx1", [128, plan1.total_ch * 8], i16,
                                         isOutput=False)
    msk1_ext = nc.declare_dram_parameter("msk1", [128, plan1.total_ch], bfl,
                                         isOutput=False)
    out_ext = nc.declare_dram_parameter("out", [NB * BLK, C2], f32, isOutput=True)

    t1_shard = nc.dram_tensor("t1_shard", [SHARD, TROW], bfl)
    t1_full = nc.dram_tensor("t1_full", [N_NODES, TROW], bfl, addr_space="Shared")
    t2_shard = nc.dram_tensor("t2_shard", [SHARD, TROW], bfl)
    t2_full = nc.dram_tensor("t2_full", [N_NODES, TROW], bfl, addr_space="Shared")

    rg = [list(range(N_CORES))]
    pieces = _piece_bounds()

    with tile.TileContext(nc) as tc:
        with tc.tile_pool(name="const", bufs=1) as cpool:
            ident = cpool.tile([128, 128], bfl)
            make_identity(nc, ident[:, :])
            a1s_t = cpool.tile([128, D1], f32)
            nc.sync.dma_start(out=a1s_t[:, :], in_=a1s_ext[:, :])
            a1d_t = cpool.tile([128, D1], f32)
            nc.sync.dma_start(out=a1d_t[:, :], in_=a1d_ext[:, :])
            a2s_t = cpool.tile([128, C2], f32)
            nc.sync.dma_start(out=a2s_t[:, :], in_=a2s_ext[:, :])
            a2d_t = cpool.tile([128, C2], f32)
            nc.sync.dma_start(out=a2d_t[:, :], in_=a2d_ext[:, :])
            b1_t = cpool.tile([128, D1], f32)
            nc.sync.dma_start(out=b1_t[:, :], in_=b1_ext[:, :])
            b2_t = cpool.tile([128, C2], f32)
            nc.sync.dma_start(out=b2_t[:, :], in_=b2_ext[:, :])
            w2_t = cpool.tile([D1, C2], bfl)
            nc.sync.dma_start(out=w2_t[:, :], in_=w2_ext[:, :])
            tab1_sb = cpool.tile([128, NB, TROW], bfl)
            tab2_sb = cpool.tile([128, NB, TROW], bfl)
            nc.vector.memset(tab1_sb[:, :, :], 0.0)
            nc.vector.memset(tab2_sb[:, :, :], 0.0)
            idx_all = cpool.tile([128, plan1.total_ch * 8], i16)
            nc.sync.dma_start(out=idx_all[:, :], in_=idx1_ext[:, :])
            msk_all = cpool.tile([128, plan1.total_ch], bfl)
            nc.sync.dma_start(out=msk_all[:, :], in_=msk1_ext[:, :])


            # ---- Phase A: h1 = x @ W1 + attention scalars, piecewise AG1
            with tc.tile_pool(name="phA", bufs=2) as apool, \
                 tc.tile_pool(name="phA_ps", bufs=4, space="PSUM") as apsum:
                w1_t = apool.tile([128, 4, D1], bfl, tag="w1")
                nc.sync.dma_start(out=w1_t[:, :, :], in_=w1_ext[:, :])
                xk = []
                for k in range(4):
                    xt = apool.tile([128, NPADROWS], bfl, tag=f"xk{k}")
                    nc.sync.dma_start(out=xt[:, :],
                                      in_=xT_ext[k * 128:(k + 1) * 128, :])
                    xk.append(xt)
                for (g0, g1, bs, be) in pieces:
                    for b in range(bs, be):
                        hps = apsum.tile([128, D1], f32, tag="hps")
                        for k in range(4):
                            nc.tensor.matmul(
                                hps[:, :],
                                lhsT=xk[k][:, b * BLK:(b + 1) * BLK],
                                rhs=w1_t[:, k, :], start=(k == 0), stop=(k == 3))
                        nc.scalar.activation(
                            out=tab1_sb[:, b, 0:D1], in_=hps[:, :],
                            func=mybir.ActivationFunctionType.Copy)
                        for a_t, sl in ((a1s_t, T1_AS), (a1d_t, T1_AD)):
                            tmp = apool.tile([128, D1], f32, tag="atmp")
                            nc.vector.tensor_tensor(
                                out=tmp[:, :], in0=hps[:, :], in1=a_t[:, :],
                                op=mybir.AluOpType.mult)
                            nc.vector.tensor_reduce(
                                out=tab1_sb[:, b, sl[0]:sl[1]].bitcast(f32),
                                in_=tmp[:, :].rearrange(
                                    "p (h c) -> p h c", h=H1, c=HID),
                                axis=mybir.AxisListType.X,
                                op=mybir.AluOpType.add)
                    _dma_blocks_out(nc, t1_shard, tab1_sb, bs, be)
                    _ag_piece(nc, t1_shard, t1_full, bs, be, T1_AGC, rg)

            _ag_whole(nc, t1_shard, t1_full, rg)

            _edge_phase(nc, tc, layer=1, table_full=t1_full,
                        idx_all=idx_all, msk_all=msk_all, ident=ident,
                        plan=plan1, pieces=pieces, bias=b1_t, w2_t=w2_t,
                        a2s_t=a2s_t, a2d_t=a2d_t, tab_sb=tab1_sb,
                        tab_out=tab2_sb, t_shard=t2_shard, t_full=t2_full,
                        rg=rg, out_ext=None, b2_t=None)

            _ag_whole(nc, t2_shard, t2_full, rg)

            _edge_phase(nc, tc, layer=2, table_full=t2_full,
                        idx_all=idx_all, msk_all=msk_all, ident=ident,
                        plan=plan2, pieces=pieces, bias=None, w2_t=None,
                        a2s_t=None, a2d_t=None, tab_sb=tab2_sb,
                        tab_out=None, t_shard=None, t_full=None,
                        rg=rg, out_ext=out_ext, b2_t=b2_t)

    nc.compile()
    return nc


def _edge_phase(nc, tc, layer, table_full, idx_all, msk_all, ident,
                plan, pieces, bias, w2_t, a2s_t, a2d_t, tab_sb,
                tab_out, t_shard, t_full, rg, out_ext, b2_t):
    if layer == 1:
        NH, CH, CC = H1, HID, D1
        asrc_sl, adst_sl = T1_AS, T1_AD
    else:
        NH, CH, CC = 1, C2, C2
        asrc_sl, adst_sl = T2_AS, T2_AD
    NCOL = CC + NH
    gmax = max(g["gch"] for g in plan.groups)

    with tc.tile_pool(name=f"self{layer}", bufs=1) as spool, \
         tc.tile_pool(name=f"e{layer}_g", bufs=3) as gpool, \
         tc.tile_pool(name=f"e{layer}", bufs=2) as pool, \
         tc.tile_pool(name=f"e{layer}_ps", bufs=2, space="PSUM") as psum, \
         tc.tile_pool(name=f"e{layer}_ps2", bufs=2, space="PSUM") as psum2:
        # analytic self-loop term: w = exp(leakyrelu(a_src[d]+a_dst[d])),
        # num += w*h[d], den += w  (self-loops excluded from edge streams)
        ws = spool.tile([128, NB, NH], f32)
        sn = spool.tile([128, NB, CC], bfl)
        with tc.tile_pool(name=f"selftmp{layer}", bufs=1) as tpool:
            es = tpool.tile([128, NB, NH], f32)
            nc.vector.tensor_tensor(
                out=es[:, :, :],
                in0=tab_sb[:, :, asrc_sl[0]:asrc_sl[1]].bitcast(f32),
                in1=tab_sb[:, :, adst_sl[0]:adst_sl[1]].bitcast(f32),
                op=mybir.AluOpType.add)
            lrs = tpool.tile([128, NB, NH], f32)
            nc.vector.scalar_tensor_tensor(
                out=lrs[:, :, :], in0=es[:, :, :], scalar=NEG_SLOPE,
                in1=es[:, :, :], op0=mybir.AluOpType.mult,
                op1=mybir.AluOpType.max)
            nc.scalar.activation(out=ws[:, :, :], in_=lrs[:, :, :],
                                 func=mybir.ActivationFunctionType.Exp)
        nc.vector.tensor_tensor(
            out=sn[:, :, :].rearrange("p b (h c) -> p b h c", h=NH, c=CH),
            in0=tab_sb[:, :, 0:CC].rearrange("p b (h c) -> p b h c",
                                             h=NH, c=CH),
            in1=ws[:, :, :, None].to_broadcast([128, NB, NH, CH]),
            op=mybir.AluOpType.mult)
        piece_of_group = {}
        for pi, (g0, g1, bs, be) in enumerate(pieces):
            for g in range(g0, g1):
                piece_of_group[g] = pi if g == g1 - 1 else None

        for gi, grp in enumerate(plan.groups):
            goff, gch = grp["goff"], grp["gch"]
            mskg = msk_all[:, goff:goff + gch]
            g_t = gpool.tile([128, gmax, TROW], bfl, tag="gath")
            for (c0, n, base, q) in grp["segs"]:
                in_ap = table_full[BASES[base]:BASES[base] + SPLIT, :]
                nc.gpsimd.dma_gather(
                    out_ap=g_t[:, c0:c0 + n, :], in_ap=in_ap,
                    idxs_ap=idx_all[:, (goff + c0) * 8:(goff + c0 + n) * 8],
                    num_idxs=n * BLK, num_idxs_reg=n * BLK,
                    elem_size=TROW, single_packet=False, queue_num=q)

            # group-wide attention pipeline
            e_t = pool.tile([128, gmax, NH], f32, tag="elog")
            for b, (l0, rn) in zip(grp["blocks"], grp["runs"]):
                nc.vector.tensor_tensor(
                    out=e_t[:, l0:l0 + rn, :],
                    in0=g_t[:, l0:l0 + rn,
                            asrc_sl[0]:asrc_sl[1]].bitcast(f32),
                    in1=tab_sb[:, b, adst_sl[0]:adst_sl[1]].bitcast(f32)
                        [:, None, :].to_broadcast([128, rn, NH]),
                    op=mybir.AluOpType.add)
            lr_t = pool.tile([128, gmax, NH], f32, tag="lrt")
            nc.vector.scalar_tensor_tensor(
                out=lr_t[:, 0:gch, :], in0=e_t[:, 0:gch, :], scalar=NEG_SLOPE,
                in1=e_t[:, 0:gch, :], op0=mybir.AluOpType.mult,
                op1=mybir.AluOpType.max)
            exf = pool.tile([128, gmax, NH], f32, tag="exf")
            nc.scalar.activation(out=exf[:, 0:gch, :], in_=lr_t[:, 0:gch, :],
                                 func=mybir.ActivationFunctionType.Exp)
            r_t = pool.tile([128, gmax, NCOL], bfl, tag="rmat")
            nc.vector.tensor_tensor(
                out=r_t[:, 0:gch, CC:NCOL], in0=exf[:, 0:gch, :],
                in1=mskg[:, :, None].to_broadcast([128, gch, NH]),
                op=mybir.AluOpType.mult)
            nc.vector.tensor_tensor(
                out=r_t[:, 0:gch, 0:CC].rearrange("p g (h c) -> p g h c",
                                                  h=NH, c=CH),
                in0=g_t[:, 0:gch, 0:CC].rearrange("p g (h c) -> p g h c",
                                                  h=NH, c=CH),
                in1=r_t[:, 0:gch, CC:NCOL, None].to_broadcast(
                    [128, gch, NH, CH]),
                op=mybir.AluOpType.mult)

            for b, (l0, rn) in zip(grp["blocks"], grp["runs"]):
                ps = psum.tile([128, NCOL], f32, tag="agg")
                for j in range(rn):
                    nc.tensor.matmul(ps[:, :], lhsT=ident[:, :],
                                     rhs=r_t[:, l0 + j, :], start=(j == 0),
                                     stop=(j == rn - 1))
                den = pool.tile([128, NH], f32, tag="den")
                nc.vector.scalar_tensor_tensor(
                    out=den[:, :], in0=ps[:, CC:NCOL], scalar=1e-16,
                    in1=ws[:, b, :], op0=mybir.AluOpType.add,
                    op1=mybir.AluOpType.add)
                recip = pool.tile([128, NH], f32, tag="recip")
                nc.vector.reciprocal(out=recip[:, :], in_=den[:, :])
                onum = pool.tile([128, CC], f32, tag="onum")
                nc.vector.tensor_tensor(out=onum[:, :], in0=ps[:, 0:CC],
                                        in1=sn[:, b, :],
                                        op=mybir.AluOpType.add)
                o_t = pool.tile([128, CC], f32, tag="outb")
                nc.vector.tensor_tensor(
                    out=o_t[:, :].rearrange("p (h c) -> p h c", h=NH, c=CH),
                    in0=onum[:, :].rearrange("p (h c) -> p h c", h=NH, c=CH),
                    in1=recip[:, :, None].to_broadcast([128, NH, CH]),
                    op=mybir.AluOpType.mult)

                if layer == 1:
                    obt = pool.tile([128, CC], f32, tag="outbt")
                    nc.vector.tensor_tensor(out=obt[:, :], in0=o_t[:, :],
                                            in1=bias[:, :],
                                            op=mybir.AluOpType.add)
                    ob = pool.tile([128, CC], bfl, tag="outbf")
                    nc.vector.tensor_scalar(out=ob[:, :], in0=obt[:, :],
                                            scalar1=0.0, scalar2=None,
                                            op0=mybir.AluOpType.max)
                    tps = psum2.tile([D1, 128], bfl, tag="tp")
                    nc.tensor.transpose(tps[:, :], ob[:, :], ident[:, :])
                    h1T = pool.tile([D1, 128], bfl, tag="h1T")
                    nc.vector.tensor_copy(out=h1T[:, :], in_=tps[:, :])
                    h2ps = psum2.tile([128, C2], f32, tag="h2")
                    nc.tensor.matmul(h2ps[:, :], lhsT=h1T[:, :], rhs=w2_t[:, :],
                                     start=True, stop=True)
                    nc.scalar.activation(out=tab_out[:, b, 0:C2],
                                         in_=h2ps[:, :],
                                         func=mybir.ActivationFunctionType.Copy)
                    for a_t, sl in ((a2s_t, T2_AS), (a2d_t, T2_AD)):
                        t2a = pool.tile([128, C2], f32, tag="t2a")
                        nc.vector.tensor_tensor(out=t2a[:, :], in0=h2ps[:, :],
                                                in1=a_t[:, :],
                                                op=mybir.AluOpType.mult)
                        nc.vector.tensor_reduce(
                            out=tab_out[:, b, sl[0]:sl[1]].bitcast(f32),
                            in_=t2a[:, :], axis=mybir.AxisListType.X,
                            op=mybir.AluOpType.add)
                else:
                    lg = pool.tile([128, C2], f32, tag="logits")
                    nc.vector.tensor_tensor(out=lg[:, :], in0=o_t[:, :],
                                            in1=b2_t[:, :],
                                            op=mybir.AluOpType.add)
                    negm = pool.tile([128, 1], f32, tag="negm")
                    nc.vector.tensor_reduce(out=negm[:, :], in_=lg[:, :],
                                            axis=mybir.AxisListType.X,
                                            op=mybir.AluOpType.max, negate=True)
                    ex = pool.tile([128, C2], f32, tag="sfex")
                    ssum = pool.tile([128, 1], f32, tag="ssum")
                    nc.scalar.activation(out=ex[:, :], in_=lg[:, :],
                                         func=mybir.ActivationFunctionType.Exp,
                                         bias=negm[:, :], accum_out=ssum[:, :])
                    lse = pool.tile([128, 1], f32, tag="lse")
                    nc.scalar.activation(out=lse[:, :], in_=ssum[:, :],
                                         func=mybir.ActivationFunctionType.Ln)
                    res = pool.tile([128, C2], f32, tag="res")
                    nc.vector.scalar_tensor_tensor(
                        out=res[:, :], in0=lg[:, :], scalar=negm[:, :],
                        in1=lse[:, :].to_broadcast([128, C2]),
                        op0=mybir.AluOpType.add, op1=mybir.AluOpType.subtract)
                    nc.scalar.dma_start(out=out_ext[b * BLK:(b + 1) * BLK, :],
                                      in_=res[:, :])

            if layer == 1:
                pi = piece_of_group.get(gi)
                if pi is not None:
                    g0, g1, bs, be = pieces[pi]
                    _dma_blocks_out(nc, t_shard, tab_out, bs, be)
                    _ag_piece(nc, t_shard, t_full, bs, be, T2_AGC, rg)


def _host_inputs(x, W1, att_src1, att_dst1, b1, W2, att_src2, att_dst2, b2,
                 plan1, plan2):
    NPADROWS = NB * BLK
    w1r = np.ascontiguousarray(
        np.asarray(W1, np.float32).reshape(4, 128, D1).transpose(1, 0, 2)
    ).reshape(128, 4 * D1).astype(bf16)
    rep = lambda v, n: np.tile(np.asarray(v, np.float32).reshape(1, n),
                               (128, 1)).astype(np.float32)
    x32 = np.asarray(x, np.float32)

    in_maps = []
    for c in range(N_CORES):
        xs = x32[c * SHARD:(c + 1) * SHARD][plan1.perm[c, :SHARD]]
        xT = np.zeros((F_IN, NPADROWS), bf16)
        xT[:, :SHARD] = xs.T.astype(bf16)
        in_maps.append({
            "xT": xT,
            "w1r": w1r,
            "w2": np.asarray(W2, np.float32).astype(bf16),
            "a1srep": rep(att_src1, D1),
            "a1drep": rep(att_dst1, D1),
            "a2srep": rep(att_src2, C2),
            "a2drep": rep(att_dst2, C2),
            "b1rep": rep(b1, D1),
            "b2rep": rep(b2, C2),
            "idx1": plan1.idx_streams[c],
            "msk1": plan1.mask_streams[c],
        })
    return in_maps


def kernel_run(inputs, trace=False):
    edge_index = inputs["edge_index"]
    plan1, plan2 = _prep(edge_index)

    key = tuple(plan1.tot)
    if key not in _CACHE:
        _CACHE[key] = _build(plan1, plan2)
    nc = _CACHE[key]

    in_maps = _host_inputs(
        inputs["x"], inputs["W1"], inputs["att_src1"], inputs["att_dst1"],
        inputs["b1"], inputs["W2"], inputs["att_src2"], inputs["att_dst2"],
        inputs["b2"], plan1, plan2)

    if trace:
        _install_ntff_hook()
    res = run_bass_kernel_spmd(nc, in_maps, core_ids=list(range(N_CORES)),
                               trace=trace)
    out = np.zeros((N_NODES, C2), np.float32)
    for c in range(N_CORES):
        o = res.results[c]["out"]
        mem = plan2.perm[c]
        valid = mem >= 0
        out[c * SHARD + mem[valid]] = o[valid]
    return out, res.exec_time_ns


def kernel(**inputs):
    out, _ = kernel_run(inputs)
    return out
